# revision 1
# baseline (speedup 1.0000x reference)
"""GCN (3-layer, PyG GCNConv-style) forward pass on 8 Trainium2 NeuronCores.

Strategy (dst-sharded graph parallelism):
  - Nodes are partitioned contiguously across the 8 cores (2560 per core,
    tiled into 20 tiles of 128 dst slots).
  - Per layer l:  Z = dis * (H @ Wl)  computed locally per core on its node
    shard (dis = deg^-1/2 absorbs the symmetric GCN normalization:
    out[d] = dis[d] * sum_e dis[src_e] * Z[src_e]).
  - AllGather replicates Z (bf16) to every core's HBM.  Each AllGather is
    split into two halves (first/second 1280 rows of every core's shard) so
    the first half can fire while the previous layer's second half of tiles
    is still computing, hiding collective latency behind gather descriptor
    generation.
  - Each core gathers its incoming edges' source rows with the SWDGE
    dma_gather instruction (128 edges -> 128 SBUF partitions per block) and
    performs the segment-sum as a matmul with a 0/1 selector matrix
    (lhsT = selector [128 edges x 128 dst slots], rhs = messages
    [128 edges x feat]), accumulating blocks per dst tile in PSUM.
    Edges are grouped by (dst tile, src half); the half-a partial sums are
    spilled to SBUF while the half-b AllGather completes.
  - Post-ops: out = relu(dis * psum + b); the next layer's Z is computed
    immediately per tile (PE transpose -> matmul with W), keeping PE warm.

Edges (with self-loops appended) are grouped by (dst tile, src half) on the
host and padded per group to a block multiple of 128; the block counts are
maxed across cores so all 8 cores execute an identical SPMD program.
Padding rows have a zero selector column and gather row 0 (finite * 0 = 0).
"""

import os
import sys

import numpy as np

sys.path.insert(0, "/opt/trn_rl_repo")

import ml_dtypes  # noqa: E402

import concourse.bass as bass  # noqa: E402
import concourse.bacc as bacc  # noqa: E402
import concourse.mybir as mybir  # noqa: E402
from concourse.bass_utils import run_bass_kernel_spmd  # noqa: E402
from concourse.library_config import mlp as _mlp_lib  # noqa: E402
from concourse.tile import TileContext  # noqa: E402
from concourse.tile_rust import add_dep_helper  # noqa: E402

BF16 = ml_dtypes.bfloat16

# ----------------------------------------------------------------------------
# Problem configuration (hardcoded for nn_Encoder_17386027614431)
# ----------------------------------------------------------------------------
N_NODES = 20000
N_CORES = 8
T = 128          # dst slots per tile (= SBUF partitions)
NT = 20          # tiles per core
SHARD = NT * T   # 2560 node slots per core
D0 = 256                 # input feature dim
DL = [256, 128, 128]     # per-layer output dims (layer 3 padded 64 -> 128)
D3_REAL = 64
NCH = 3                  # gather chunks per half-layer


def _chunk_ranges(nt, nch):
    per = (nt + nch - 1) // nch
    return [(i, min(i + per, nt)) for i in range(0, nt, per)]


def _build_nc(BH, n_nodes=N_NODES, nt=NT, nch=NCH, d0=D0, dl=None,
              d3_real=D3_REAL):
    """Build the SPMD Bass program.

    BH: [nt][2] list - number of 128-edge blocks per (dst tile, src half),
    identical across cores."""
    if dl is None:
        dl = DL
    shard = nt * T
    ht_a = max(1, (nt * 7) // 20)   # tiles in AG half a (early trigger)
    half_a = ht_a * T
    half_b = shard - half_a
    f32 = mybir.dt.float32
    bf16 = mybir.dt.bfloat16
    i16 = mybir.dt.int16
    mult = mybir.AluOpType.mult
    add = mybir.AluOpType.add
    relu = mybir.ActivationFunctionType.Relu

    # block offsets: all half-0 groups then all half-1 groups
    boff = [[0, 0] for _ in range(nt + 1)]
    off = 0
    for h in range(2):
        for j in range(nt):
            boff[j][h] = off
            off += BH[j][h]
    totblk = off
    hb0 = sum(BH[j][0] for j in range(nt))  # blocks in half 0

    nc = bacc.Bacc("TRN2", num_devices=N_CORES)

    # ---- kernel I/O ----
    xt = nc.dram_tensor("xt", [d0, shard], bf16, kind="ExternalInput")
    w1 = nc.dram_tensor("w1", [d0, dl[0]], bf16, kind="ExternalInput")
    w2 = nc.dram_tensor("w2", [dl[0], dl[1]], bf16, kind="ExternalInput")
    w3 = nc.dram_tensor("w3", [dl[1], dl[2]], bf16, kind="ExternalInput")
    brep1 = nc.dram_tensor("brep1", [T, dl[0]], f32, kind="ExternalInput")
    brep2 = nc.dram_tensor("brep2", [T, dl[1]], f32, kind="ExternalInput")
    brep3 = nc.dram_tensor("brep3", [T, dl[2]], f32, kind="ExternalInput")
    dis = nc.dram_tensor("dis", [T, nt], f32, kind="ExternalInput")
    idx = nc.dram_tensor("idx", [T, totblk * 8], i16, kind="ExternalInput")
    sel = nc.dram_tensor("sel", [T, totblk * T], bf16, kind="ExternalInput")
    ident = nc.dram_tensor("ident", [T, T], bf16, kind="ExternalInput")
    out = nc.dram_tensor("out", [shard, d3_real], f32, kind="ExternalOutput")

    # ---- internal DRAM bounce buffers for the collectives (per layer/half)
    hsz = [half_a, half_b]
    agin = [[nc.dram_tensor(f"agin{l}_{h}", [hsz[h], dl[l]], bf16)
             for h in range(2)] for l in range(3)]
    agout = [[nc.dram_tensor(f"agout{l}_{h}", [N_CORES * hsz[h], dl[l]], bf16,
                             addr_space="Shared")
              for h in range(2)] for l in range(3)]
    rg = [list(range(N_CORES))]

    w_dram = [w1, w2, w3]
    w_chunks = [d0 // T, dl[0] // T, dl[1] // T]
    brep_dram = [brep1, brep2, brep3]

    with TileContext(nc) as tc:
        nc.gpsimd.load_library(_mlp_lib)

        with (
            tc.tile_pool(name="const", bufs=1) as cpool,
            tc.tile_pool(name="gath", bufs=3) as gpool,
            tc.tile_pool(name="selp", bufs=2) as spool,
            tc.tile_pool(name="accp", bufs=1) as apool,
            tc.tile_pool(name="hp", bufs=2) as hpool,
            tc.tile_pool(name="htp", bufs=3) as htpool,
            tc.tile_pool(name="tmp", bufs=3) as tpool,
            tc.tile_pool(name="zbp", bufs=3) as zbpool,
            tc.tile_pool(name="ps_agg", bufs=2, space="PSUM") as ps_agg,
            tc.tile_pool(name="ps_t", bufs=2, space="PSUM") as ps_t,
            tc.tile_pool(name="ps_z", bufs=2, space="PSUM") as ps_z,
        ):
            # ---- load constants ----
            def load_const(dram_h, shape, dtype, view=None):
                t = cpool.tile(shape, dtype, tag=f"c_{dram_h.name}")
                src = dram_h.ap() if view is None else view
                nc.sync.dma_start(out=t[:, :], in_=src)
                return t

            def load_const_chunked(dram_h, inner, dtype):
                cs = dram_h.shape[0] // T
                t = cpool.tile([T, cs * inner], dtype, tag=f"c_{dram_h.name}")
                nc.sync.dma_start(
                    out=t.rearrange("p (c n) -> p c n", c=cs),
                    in_=dram_h.ap().rearrange("(c p) n -> p c n", p=T),
                )
                return t

            idx_sb = load_const(idx, [T, totblk * 8], i16)
            xt_sb = load_const_chunked(xt, shard, bf16)
            w_sb = [load_const_chunked(w_dram[l], dl[l], bf16) for l in range(3)]
            brep_sb = [load_const(brep_dram[l], [T, dl[l]], f32) for l in range(3)]
            dis_sb = load_const(dis, [T, nt], f32)
            ident_sb = load_const(ident, [T, T], bf16)

            out_v = out.ap().rearrange("(n p) d -> p n d", p=T)
            agin_v = [[agin[l][h].ap().rearrange("(n p) d -> p n d", p=T)
                       for h in range(2)] for l in range(3)]

            ag_insts = [[None, None] for _ in range(3)]
            agin_dmas = [[[], []] for _ in range(3)]

            def z_prescale_store(l, j, zp):
                """dis * psum -> bf16 -> DRAM agin[l][half-of-j]."""
                zb = zbpool.tile([T, dl[l]], bf16, tag="zb")
                nc.vector.tensor_scalar(zb[:, :], zp, dis_sb[:, j:j + 1], None, mult)
                h = 0 if j < ht_a else 1
                d = nc.sync.dma_start(
                    out=agin_v[l][h][:, j - h * ht_a, :], in_=zb[:, :]
                )
                agin_dmas[l][h].append(d)

            def issue_ag(l, h):
                cc = nc.gpsimd.collective_compute(
                    "AllGather",
                    mybir.AluOpType.bypass,
                    replica_groups=rg,
                    ins=[agin[l][h].ap().opt()],
                    outs=[agout[l][h].ap().opt()],
                )
                for d in agin_dmas[l][h]:
                    add_dep_helper(cc.ins, d.ins, reason=f"ag{l}.{h} after dmas")
                ag_insts[l][h] = cc

            # ---- layer 1 local Z' = dis * (x @ W1) ----
            for j in range(nt):
                zp = ps_z.tile([T, dl[0]], f32, tag="zpsum")
                for c in range(w_chunks[0]):
                    nc.tensor.matmul(
                        zp[:, :],
                        xt_sb[:, c * shard + j * T: c * shard + (j + 1) * T],
                        w_sb[0][:, c * dl[0]:(c + 1) * dl[0]],
                        start=(c == 0),
                        stop=(c == w_chunks[0] - 1),
                    )
                z_prescale_store(0, j, zp[:, :])
                if j == ht_a - 1:
                    issue_ag(0, 0)

            # ---- aggregation layers ----
            chunks_a = _chunk_ranges(nt, 4)
            bsplit = max(1, ht_a - 3)
            chunks_b = ([(0, bsplit), (bsplit, ht_a)] if ht_a > 1
                        else [(0, ht_a)])
            chunks_b += [(a + ht_a, b + ht_a)
                         for (a, b) in _chunk_ranges(nt - ht_a, 3)]
            trig_tile = min(ht_a + 1, nt - 1)
            for l in range(3):
                d_el = dl[l]
                last = l == 2
                acc = apool.tile([T, nt * d_el], f32, tag="acc")

                def do_gather(j0, j1, h):
                    b0, b1 = boff[j0][h], boff[j1 - 1][h] + BH[j1 - 1][h]
                    nb = b1 - b0
                    gt = gpool.tile([T, nb * d_el], bf16, tag="gath")
                    gt3 = gt.rearrange("p (n d) -> p n d", d=d_el)
                    g = nc.gpsimd.dma_gather(
                        gt3,
                        agout[l][h].ap(),
                        idx_sb[:, b0 * 8:b1 * 8],
                        nb * T,
                        nb * T,
                        d_el,
                        single_packet=False,
                    )
                    add_dep_helper(g.ins, ag_insts[l][h].ins,
                                   reason=f"gather{l}.{h} after ag")
                    st = spool.tile([T, nb * T], bf16, tag="sel")
                    st3 = st.rearrange("p (n d) -> p n d", d=T)
                    nc.sync.dma_start(
                        out=st[:, :], in_=sel[:, b0 * T:b1 * T]
                    )
                    return gt3, st3, b0

                # ---- phase A: half-0 blocks -> acc ----
                for ci, (j0, j1) in enumerate(chunks_a):
                    gt3, st3, b0 = do_gather(j0, j1, 0)
                    if ci == 0:
                        issue_ag(l, 1)
                    for j in range(j0, j1):
                        ps = ps_agg.tile([T, d_el], f32, tag="aggpsum")
                        nb_j = BH[j][0]
                        jb = boff[j][0] - b0
                        for b in range(nb_j):
                            nc.tensor.matmul(
                                ps[:, :],
                                st3[:, jb + b, :],
                                gt3[:, jb + b, :],
                                start=(b == 0),
                                stop=(b == nb_j - 1),
                            )
                        nc.vector.tensor_copy(acc[:, j * d_el:(j + 1) * d_el],
                                              ps[:, :])

                # ---- phase B: half-1 blocks + post-ops ----
                for (j0, j1) in chunks_b:
                    gt3, st3, b0 = do_gather(j0, j1, 1)
                    for j in range(j0, j1):
                        ps = ps_agg.tile([T, d_el], f32, tag="aggpsum")
                        nb_j = BH[j][1]
                        jb = boff[j][1] - b0
                        for b in range(nb_j):
                            nc.tensor.matmul(
                                ps[:, :],
                                st3[:, jb + b, :],
                                gt3[:, jb + b, :],
                                start=(b == 0),
                                stop=(b == nb_j - 1),
                            )
                        # post: s = psB + acc; t = s*dis; u = t + b; relu
                        s = tpool.tile([T, d_el], f32, tag="post0")
                        nc.vector.tensor_tensor(
                            s[:, :], ps[:, :], acc[:, j * d_el:(j + 1) * d_el],
                            add)
                        t1 = tpool.tile([T, d_el], f32, tag="post1")
                        nc.vector.tensor_scalar(
                            t1[:, :], s[:, :], dis_sb[:, j:j + 1], None, mult)
                        t2 = tpool.tile([T, d_el], f32, tag="post2")
                        nc.vector.tensor_tensor(
                            t2[:, :], t1[:, :], brep_sb[l][:, :], add)
                        if last:
                            nc.sync.dma_start(
                                out=out_v[:, j, :], in_=t2[:, :d3_real])
                        else:
                            h = hpool.tile([T, d_el], bf16, tag="h")
                            nc.scalar.activation(h[:, :], t2[:, :], relu)
                            ln = l + 1
                            cs = w_chunks[ln]
                            zp = ps_z.tile([T, dl[ln]], f32, tag="zpsum")
                            for c in range(cs):
                                tp = ps_t.tile([T, T], bf16, tag="tpsum")
                                nc.tensor.matmul(
                                    tp[:, :],
                                    h[:, c * T:(c + 1) * T],
                                    ident_sb[:, :],
                                    is_transpose=True,
                                )
                                htc = htpool.tile([T, T], bf16, tag="ht")
                                nc.vector.tensor_copy(htc[:, :], tp[:, :])
                                nc.tensor.matmul(
                                    zp[:, :],
                                    htc[:, :],
                                    w_sb[ln][:, c * dl[ln]:(c + 1) * dl[ln]],
                                    start=(c == 0),
                                    stop=(c == cs - 1),
                                )
                            z_prescale_store(ln, j, zp[:, :])
                            if j == trig_tile:
                                issue_ag(ln, 0)

    nc.compile()
    return nc


# ----------------------------------------------------------------------------
# Host-side preprocessing (index work + sharding)
# ----------------------------------------------------------------------------
def _balanced_node_order(deg, n_nodes, nt):
    """Assign nodes to (core, tile) buckets so per-bucket in-edge counts are
    near-equal: sort by degree desc, deal round-robin (snake) over buckets.
    Returns node_order[n_slots] (original node id per slot, -1 for pad) and
    new_pos[n_nodes] (slot of each node)."""
    n_buckets = N_CORES * nt
    slots_total = n_buckets * T
    by_deg = np.argsort(-deg, kind="stable")
    node_order = -np.ones(slots_total, np.int64)
    new_pos = np.zeros(n_nodes, np.int64)
    fill = np.zeros(n_buckets, np.int64)
    b = 0
    direction = 1
    for node in by_deg:
        node_order[b * T + fill[b]] = node
        new_pos[node] = b * T + fill[b]
        fill[b] += 1
        b += direction
        if b == n_buckets:
            b = n_buckets - 1
            direction = -1
        elif b < 0:
            b = 0
            direction = 1
    # slot index within bucket -> global slot: bucket*T + k; convert to the
    # (core, tile, slot) flat layout used by the kernel (bucket = core*nt+tile)
    return node_order, new_pos


def _preprocess(edge_index, n_nodes=N_NODES, nt=NT):
    """Group (self-loop-augmented) edges by (dst tile, src half) per core;
    pad each group to a multiple of 128, block counts maxed across cores.
    Returns per-core gather indices, selectors, dis, BH[nt][2], node_order."""
    shard = nt * T
    ht_a = max(1, (nt * 7) // 20)
    half_a = ht_a * T
    half_b = shard - half_a
    src = np.asarray(edge_index[0], dtype=np.int64)
    dst = np.asarray(edge_index[1], dtype=np.int64)
    loop = np.arange(n_nodes, dtype=np.int64)
    src = np.concatenate([src, loop])
    dst = np.concatenate([dst, loop])

    deg = np.bincount(dst, minlength=n_nodes).astype(np.float64)
    dis_full = np.where(deg > 0, 1.0 / np.sqrt(deg), 0.0)

    node_order, new_pos = _balanced_node_order(deg, n_nodes, nt)

    dpos = new_pos[dst]
    spos = new_pos[src]
    core_of = dpos // shard
    tile_of = (dpos % shard) // T
    slot_of = dpos % T
    half_of = ((spos % shard) >= half_a).astype(np.int64)  # 0 or 1
    # row index within the half's gathered buffer
    hsz = np.array([half_a, half_b])
    row_of = ((spos // shard) * hsz[half_of] + (spos % shard)
              - half_of * half_a)

    counts = np.zeros((N_CORES, nt, 2), np.int64)
    np.add.at(counts, (core_of, tile_of, half_of), 1)
    bh = np.maximum(
        1, np.ceil(counts.max(axis=0) / T).astype(np.int64))  # [nt, 2]
    BH = bh.tolist()

    # block offsets (half-major), same as the builder
    boff = np.zeros((nt, 2), np.int64)
    off = 0
    for h in range(2):
        for j in range(nt):
            boff[j][h] = off
            off += bh[j][h]
    totblk = int(off)

    order = np.lexsort((tile_of, half_of, core_of))
    row_s = row_of[order]
    core_s = core_of[order]
    tile_s = tile_of[order]
    slot_s = slot_of[order]
    half_s = half_of[order]

    grp = (core_s * 2 + half_s) * nt + tile_s
    grp_start = np.zeros(N_CORES * 2 * nt + 1, np.int64)
    np.add.at(grp_start, grp + 1, 1)
    grp_start = np.cumsum(grp_start)
    rank = np.arange(len(grp)) - grp_start[grp]

    pos = boff[tile_s, half_s] * T + rank  # padded position within the core
    blk = pos // T
    lane = pos % T

    idx_cores, sel_cores, dis_cores = [], [], []
    KC = totblk * T
    for c in range(N_CORES):
        m = core_s == c
        idx_pad = np.zeros(KC, np.int16)
        idx_pad[pos[m]] = row_s[m].astype(np.int16)
        idx_wrapped = np.tile(
            idx_pad.reshape(KC // 16, 16).T, (8, 1)).astype(np.int16)
        idx_cores.append(np.ascontiguousarray(idx_wrapped))

        selc = np.zeros((totblk, T, T), np.float32)
        selc[blk[m], lane[m], slot_s[m]] = 1.0
        sel_cores.append(
            np.ascontiguousarray(
                selc.transpose(1, 0, 2).reshape(T, totblk * T)).astype(BF16))

        slots = node_order[c * shard:(c + 1) * shard]
        dis_c = np.where(slots >= 0, dis_full[np.maximum(slots, 0)], 0.0)
        dis_cores.append(
            np.ascontiguousarray(dis_c.reshape(nt, T).T).astype(np.float32))

    return idx_cores, sel_cores, dis_cores, BH, node_order


def _make_in_maps(x, W1, b1, W2, b2, W3, b3, edge_index,
                  n_nodes=N_NODES, nt=NT, d0=D0, dl=None, d3_real=D3_REAL):
    if dl is None:
        dl = DL
    shard = nt * T
    idx_cores, sel_cores, dis_cores, BH, node_order = _preprocess(
        edge_index, n_nodes, nt)

    x = np.asarray(x, np.float32)
    W3p = np.zeros((dl[1], dl[2]), np.float32)
    W3p[:, :d3_real] = np.asarray(W3, np.float32)
    b3p = np.zeros(dl[2], np.float32)
    b3p[:d3_real] = np.asarray(b3, np.float32)

    w1b = np.asarray(W1, np.float32).astype(BF16)
    w2b = np.asarray(W2, np.float32).astype(BF16)
    w3b = W3p.astype(BF16)
    brep1 = np.tile(np.asarray(b1, np.float32), (T, 1))
    brep2 = np.tile(np.asarray(b2, np.float32), (T, 1))
    brep3 = np.tile(b3p, (T, 1))
    identity = np.eye(T, dtype=BF16)

    in_maps = []
    for c in range(N_CORES):
        slots = node_order[c * shard:(c + 1) * shard]
        xs = np.where((slots >= 0)[:, None], x[np.maximum(slots, 0)], 0.0)
        xs = xs.astype(np.float32)
        in_maps.append({
            "xt": np.ascontiguousarray(xs.T).astype(BF16),
            "w1": w1b, "w2": w2b, "w3": w3b,
            "brep1": brep1, "brep2": brep2, "brep3": brep3,
            "dis": dis_cores[c],
            "idx": idx_cores[c],
            "sel": sel_cores[c],
            "ident": identity,
        })
    return in_maps, BH, node_order


_NC_CACHE = {}


def kernel_with_results(x, W1, b1, W2, b2, W3, b3, edge_index, trace=False):
    in_maps, BH, node_order = _make_in_maps(
        x, W1, b1, W2, b2, W3, b3, edge_index)
    key = tuple(tuple(r) for r in BH)
    if key not in _NC_CACHE:
        _NC_CACHE[key] = _build_nc(BH)
    nc = _NC_CACHE[key]
    res = run_bass_kernel_spmd(
        nc, in_maps, core_ids=list(range(N_CORES)), trace=trace
    )
    rows = np.concatenate(
        [np.asarray(res.results[c]["out"]) for c in range(N_CORES)], axis=0)
    full = np.zeros((N_NODES, rows.shape[1]), np.float32)
    real = node_order >= 0
    full[node_order[real]] = rows[real]
    return full, res


def kernel(x, W1, b1, W2, b2, W3, b3, edge_index):
    full, _ = kernel_with_results(x, W1, b1, W2, b2, W3, b3, edge_index)
    return full



# revision 9
# speedup vs baseline: 1.5812x; 1.5812x over previous
"""GCN (3-layer, PyG GCNConv-style) forward pass on 8 Trainium2 NeuronCores.

Architecture v2 (gather L1 + PE-scatter L2/L3):
  - Nodes partitioned contiguously across 8 cores (2560 slots each, 20 dst
    tiles of 128).  Z_l = dis * (H @ Wl) computed per core, AllGathered
    (bf16, in two halves for overlap) so each core holds the full 20480-row
    table in HBM.
  - Layer 1 aggregation uses the SWDGE dma_gather + selector-matmul path
    (gather is ~9 ns/row of GpSimd time regardless of row width, so it is
    used only for the widest layer).  Self-loops are added via an identity
    matmul on the locally-computed Z tile instead of being gathered, and
    gather calls use exact per-group index counts (no 128-padding rows).
    Selectors are fp8 (exact 0/1) streamed from HBM.
  - Layers 2/3 aggregate on the TensorEngine: for each of the 160 source
    tiles, Z_s [128 src x d] is the stationary operand and a 0/1 adjacency
    slice S_s [128 src x 2560 dst] (fp8, includes self-loops) streams as the
    moving operand, accumulating a feature-major PSUM [d x 2560] across all
    source tiles (start at s=0, stop at s=159) in five 512-column bank
    chunks.  Post-ops run feature-major (dis is a replicated row tensor,
    bias is per-partition), and the next layer's GEMM consumes H_fm directly
    as the stationary operand -- no transposes.  The final output is
    transposed back to node-major via the PE.
  - Scatter steps for source tiles in AllGather half-a are interleaved into
    the tail of the layer-1 phase-B loop so the PE works while GpSimd is
    still gathering.
"""

import sys

import numpy as np

sys.path.insert(0, "/opt/trn_rl_repo")

import ml_dtypes  # noqa: E402

import concourse.bass as bass  # noqa: E402
import concourse.bacc as bacc  # noqa: E402
import concourse.mybir as mybir  # noqa: E402
from concourse.bass_utils import run_bass_kernel_spmd  # noqa: E402
from concourse.library_config import mlp as _mlp_lib  # noqa: E402
from concourse.tile import TileContext  # noqa: E402
from concourse.tile_rust import add_dep_helper  # noqa: E402

BF16 = ml_dtypes.bfloat16
FP8 = ml_dtypes.float8_e4m3

# ----------------------------------------------------------------------------
# Problem configuration (hardcoded for nn_Encoder_17386027614431)
# ----------------------------------------------------------------------------
N_NODES = 20000
N_CORES = 8
T = 128
NT = 20                  # dst tiles per core
SHARD = NT * T           # 2560
NTAB = N_CORES * SHARD   # 20480 table rows
D0 = 256
DL = [256, 128, 64]      # per-layer output dims
HT_A = 7                 # tiles in AllGather half a
HALF_A = HT_A * T        # 896
HALF_B = SHARD - HALF_A  # 1664
HSZ = [HALF_A, HALF_B]
SA_TILES = N_CORES * HT_A          # 56 src tiles in half-a table
S_TILES = NTAB // T                # 160
CCHUNK = 512                       # psum bank columns (f32)
NCH = SHARD // CCHUNK              # 5 feature-major column chunks


def _ru16(x):
    return (int(x) + 15) // 16 * 16


def _build_nc(CNT, apply_b1, apply_b3):
    """CNT: [2][NT] exact gather index counts (multiples of 16), same on
    every core."""
    f32 = mybir.dt.float32
    bf16 = mybir.dt.bfloat16
    fp8 = mybir.dt.float8e4
    i16 = mybir.dt.int16
    mult = mybir.AluOpType.mult
    add = mybir.AluOpType.add
    relu = mybir.ActivationFunctionType.Relu
    fcopy = mybir.ActivationFunctionType.Copy

    nbl = [[(CNT[h][j] + T - 1) // T for j in range(NT)] for h in range(2)]
    boff = [[0] * NT for _ in range(2)]
    ioff = [[0] * NT for _ in range(2)]
    ob = oi = 0
    for h in range(2):
        for j in range(NT):
            boff[h][j] = ob
            ioff[h][j] = oi
            ob += nbl[h][j]
            oi += CNT[h][j] // 16
    totblk = ob
    idxcols = oi
    maxnb = max(max(nbl[0]), max(nbl[1]))

    nc = bacc.Bacc("TRN2", num_devices=N_CORES, num_swdge_queues=4)

    # ---- kernel I/O ----
    xt = nc.dram_tensor("xt", [D0, SHARD], bf16, kind="ExternalInput")
    w1 = nc.dram_tensor("w1", [D0, DL[0]], bf16, kind="ExternalInput")
    w2 = nc.dram_tensor("w2", [DL[0], DL[1]], bf16, kind="ExternalInput")
    w3 = nc.dram_tensor("w3", [DL[1], DL[2]], bf16, kind="ExternalInput")
    brep1 = nc.dram_tensor("brep1", [T, DL[0]], f32, kind="ExternalInput")
    b2col = nc.dram_tensor("b2col", [T, 1], f32, kind="ExternalInput")
    b3col = nc.dram_tensor("b3col", [T, 1], f32, kind="ExternalInput")
    dis = nc.dram_tensor("dis", [T, NT], f32, kind="ExternalInput")
    disrow = nc.dram_tensor("disrow", [T, SHARD], f32, kind="ExternalInput")
    identb = nc.dram_tensor("identb", [T, T], bf16, kind="ExternalInput")
    identf = nc.dram_tensor("identf", [T, T], f32, kind="ExternalInput")
    idx = nc.dram_tensor("idx", [T, idxcols], i16, kind="ExternalInput")
    sel = nc.dram_tensor("sel", [T, totblk * T], fp8, kind="ExternalInput")
    smat = nc.dram_tensor("smat", [NTAB, SHARD], fp8, kind="ExternalInput")
    out = nc.dram_tensor("out", [SHARD, DL[2]], f32, kind="ExternalOutput")

    # ---- internal DRAM for collectives ----
    agin = [[nc.dram_tensor(f"agin{l}_{h}", [HSZ[h], DL[l]], bf16)
             for h in range(2)] for l in range(3)]
    agout = [[nc.dram_tensor(f"agout{l}_{h}", [N_CORES * HSZ[h], DL[l]], bf16,
                             addr_space="Shared")
              for h in range(2)] for l in range(3)]
    rg = [list(range(N_CORES))]

    with TileContext(nc) as tc:
        nc.gpsimd.load_library(_mlp_lib)

        with (
            tc.tile_pool(name="const", bufs=1) as cpool,
            tc.tile_pool(name="sb", bufs=3) as sbpool,        # S stream
            tc.tile_pool(name="zsb", bufs=2) as zspool,       # Z stationary
            tc.tile_pool(name="selp", bufs=3) as selpool,
            tc.tile_pool(name="hp", bufs=2) as hpool,
            tc.tile_pool(name="htp", bufs=3) as htpool,
            tc.tile_pool(name="tmp", bufs=3) as tpool,
            tc.tile_pool(name="zbp", bufs=3) as zbpool,
            tc.tile_pool(name="ps_z", bufs=1, space="PSUM") as ps_z,
            tc.tile_pool(name="ps_agg", bufs=1, space="PSUM") as ps_agg,
            tc.tile_pool(name="ps_t", bufs=1, space="PSUM") as ps_t,
            tc.tile_pool(name="ps_fm", bufs=1, space="PSUM") as ps_fm,
        ):
            # ---- constants ----
            def load_const(dram_h, shape, dtype):
                t = cpool.tile(shape, dtype, tag=f"c_{dram_h.name}")
                nc.sync.dma_start(out=t[:, :], in_=dram_h.ap())
                return t

            def load_const_chunked(dram_h, inner, dtype):
                cs = dram_h.shape[0] // T
                t = cpool.tile([T, cs * inner], dtype, tag=f"c_{dram_h.name}")
                nc.sync.dma_start(
                    out=t.rearrange("p (c n) -> p c n", c=cs),
                    in_=dram_h.ap().rearrange("(c p) n -> p c n", p=T),
                )
                return t

            idx_sb = load_const(idx, [T, idxcols], i16)
            xt_sb = load_const_chunked(xt, SHARD, bf16)
            w1_sb = load_const_chunked(w1, DL[0], bf16)
            w2_sb = load_const_chunked(w2, DL[1], bf16)
            w3_sb = load_const(w3, [DL[1], DL[2]], bf16)
            brep1_sb = load_const(brep1, [T, DL[0]], f32)
            b2_sb = load_const(b2col, [T, 1], f32)
            b3_sb = load_const(b3col, [T, 1], f32)
            dis_sb = load_const(dis, [T, NT], f32)
            disrow_sb = load_const(disrow, [T, SHARD], f32)
            identb_sb = load_const(identb, [T, T], bf16)
            identf_sb = load_const(identf, [T, T], f32)

            # persistent buffers
            gbuf = [cpool.tile([T, maxnb * DL[0]], bf16, tag=f"g{i}",
                                name=f"gbuf{i}") for i in range(3)]
            for g in gbuf:
                nc.gpsimd.memset(g[:, :], 0.0)
            zb1 = [cpool.tile([T, DL[0]], bf16, tag=f"zb1_{j}",
                               name=f"zb1_{j}") for j in range(NT)]
            acc = cpool.tile([T, NT * DL[0]], f32, tag="acc")
            h2fm = cpool.tile([T, SHARD], bf16, tag="h2fm")
            outfm = cpool.tile([T, SHARD], f32, tag="outfm")
            fm = [ps_fm.tile([T, CCHUNK], f32, tag=f"fm{c}",
                              name=f"fm{c}") for c in range(NCH)]

            agin_v = [[agin[l][h].ap().rearrange("(n p) d -> p n d", p=T)
                       for h in range(2)] for l in range(3)]
            agouta_v = [agout[l][0].ap().rearrange("(n p) d -> p n d", p=T)
                        for l in range(3)]
            agoutb_v = [agout[l][1].ap().rearrange("(n p) d -> p n d", p=T)
                        for l in range(3)]
            smat_v = smat.ap().rearrange("(s p) d -> p s d", p=T)
            out_v = out.ap().rearrange("(n p) d -> p n d", p=T)

            ag_insts = [[None, None] for _ in range(3)]
            agin_dmas = [[[], []] for _ in range(3)]

            def z_store(l, j, zb):
                h = 0 if j < HT_A else 1
                d = nc.sync.dma_start(
                    out=agin_v[l][h][:, j - h * HT_A, :], in_=zb[:, :])
                agin_dmas[l][h].append(d)

            def issue_ag(l, h):
                cc = nc.gpsimd.collective_compute(
                    "AllGather",
                    mybir.AluOpType.bypass,
                    replica_groups=rg,
                    ins=[agin[l][h].ap().opt()],
                    outs=[agout[l][h].ap().opt()],
                )
                for d in agin_dmas[l][h]:
                    add_dep_helper(cc.ins, d.ins, reason=f"ag{l}.{h}")
                ag_insts[l][h] = cc

            # ================= Layer 1: Z1 = dis * (x @ W1) ================
            for j in range(NT):
                zp = ps_z.tile([T, DL[0]], f32, tag="zp")
                for c in range(2):
                    nc.tensor.matmul(
                        zp[:, :],
                        xt_sb[:, c * SHARD + j * T: c * SHARD + (j + 1) * T],
                        w1_sb[:, c * DL[0]:(c + 1) * DL[0]],
                        start=(c == 0), stop=(c == 1),
                    )
                nc.vector.tensor_scalar(
                    zb1[j][:, :], zp[:, :], dis_sb[:, j:j + 1], None, mult)
                z_store(0, j, zb1[j])
                if j == HT_A - 1:
                    issue_ag(0, 0)
            issue_ag(0, 1)

            # ---- gather helper ----
            gq = [0]

            def gather_group(h, j, gslot):
                cnt = CNT[h][j]
                nb = nbl[h][j]
                gt3 = gbuf[gslot][:, :nb * DL[0]].rearrange(
                    "p (n d) -> p n d", d=DL[0])
                g = nc.gpsimd.dma_gather(
                    gt3,
                    agout[0][h].ap(),
                    idx_sb[:, ioff[h][j]:ioff[h][j] + cnt // 16],
                    cnt, cnt, DL[0],
                    single_packet=False,
                    queue_num=gq[0] % 4,
                )
                gq[0] += 1
                add_dep_helper(g.ins, ag_insts[0][h].ins, reason="g after ag")
                st = selpool.tile([T, maxnb * T], fp8, tag="sel")
                nc.sync.dma_start(
                    out=st[:, :nb * T],
                    in_=sel[:, boff[h][j] * T:(boff[h][j] + nb) * T])
                return gt3, st, nb

            # ================= L1 aggregation phase A (half 0) =============
            for j in range(NT):
                gt3, st, nb = gather_group(0, j, j % 3)
                ps = ps_agg.tile([T, DL[0]], f32, tag="agg")
                nc.tensor.matmul(ps[:, :], identb_sb[:, :], zb1[j][:, :],
                                 start=True, stop=False)
                for b in range(nb):
                    nc.tensor.matmul(
                        ps[:, :], st[:, b * T:(b + 1) * T], gt3[:, b, :],
                        start=False, stop=(b == nb - 1))
                nc.scalar.activation(
                    acc[:, j * DL[0]:(j + 1) * DL[0]], ps[:, :], fcopy)

            # ---- L2 scatter machinery (interleaved into phase B tail) ----
            scat_state = {"s": 0, "zsb": None}

            def scatter_steps(l, n, d_el, limit=S_TILES):
                st_ = scat_state
                for _ in range(n):
                    s = st_["s"]
                    if s >= limit:
                        return
                    if s % 8 == 0:
                        zsb = zspool.tile([T, 8 * d_el], bf16, tag=f"zs{l}")
                        if s < SA_TILES:
                            src = agouta_v[l][:, s:s + 8, :]
                            agdep = ag_insts[l][0]
                        else:
                            src = agoutb_v[l][:, s - SA_TILES:s - SA_TILES + 8, :]
                            agdep = ag_insts[l][1]
                        d = nc.sync.dma_start(
                            out=zsb.rearrange("p (n d) -> p n d", d=d_el),
                            in_=src)
                        add_dep_helper(d.ins, agdep.ins, reason="zs after ag")
                        st_["zsb"] = zsb
                    stile = sbpool.tile([T, SHARD], fp8, tag="smat")
                    nc.sync.dma_start(out=stile[:, :], in_=smat_v[:, s, :])
                    zsb = st_["zsb"]
                    k = s % 8
                    for c in range(NCH):
                        nc.tensor.matmul(
                            fm[c][:d_el, :],
                            zsb[:, k * d_el:(k + 1) * d_el],
                            stile[:, c * CCHUNK:(c + 1) * CCHUNK],
                            start=(s == 0), stop=(s == S_TILES - 1))
                    st_["s"] = s + 1

            # ================= L1 phase B + post + Z2 + AG2 ================
            for j in range(NT):
                gt3, st, nb = gather_group(1, j, j % 3)
                ps = ps_agg.tile([T, DL[0]], f32, tag="agg")
                for b in range(nb):
                    nc.tensor.matmul(
                        ps[:, :], st[:, b * T:(b + 1) * T], gt3[:, b, :],
                        start=(b == 0), stop=(b == nb - 1))
                u = tpool.tile([T, DL[0]], f32, tag="post")
                nc.vector.tensor_tensor(
                    u[:, :], ps[:, :], acc[:, j * DL[0]:(j + 1) * DL[0]], add)
                if apply_b1:
                    u2 = tpool.tile([T, DL[0]], f32, tag="post")
                    nc.vector.tensor_tensor(u2[:, :], u[:, :], brep1_sb[:, :],
                                            add)
                    u = u2
                h1 = hpool.tile([T, DL[0]], bf16, tag="h1")
                nc.scalar.activation(h1[:, :], u[:, :], relu,
                                     scale=dis_sb[:, j:j + 1])
                zp2 = ps_z.tile([T, DL[0]], f32, tag="zp")
                for c in range(2):
                    tp = ps_t.tile([T, T], bf16, tag="tp")
                    nc.tensor.matmul(tp[:, :], h1[:, c * T:(c + 1) * T],
                                     identb_sb[:, :], is_transpose=True)
                    htc = htpool.tile([T, T], bf16, tag="ht")
                    nc.scalar.activation(htc[:, :], tp[:, :], fcopy)
                    nc.tensor.matmul(
                        zp2[:, :DL[1]], htc[:, :],
                        w2_sb[:, c * DL[1]:(c + 1) * DL[1]],
                        start=(c == 0), stop=(c == 1))
                zb2 = zbpool.tile([T, DL[1]], bf16, tag="zb2")
                nc.vector.tensor_scalar(
                    zb2[:, :], zp2[:, :DL[1]], dis_sb[:, j:j + 1], None, mult)
                z_store(1, j, zb2)
                if j == HT_A - 1:
                    issue_ag(1, 0)
                if j == NT - 1:
                    issue_ag(1, 1)
                if j >= HT_A:
                    scatter_steps(1, 5, DL[1], limit=SA_TILES)

            # ---- L2 scatter remainder ----
            scatter_steps(1, S_TILES, DL[1])

            # ---- L2 post (feature-major) + Z3 + AG3 ----
            for c in range(NCH):
                t = tpool.tile([T, CCHUNK], f32, tag="fmpost")
                nc.vector.tensor_tensor(
                    t[:, :], fm[c][:, :],
                    disrow_sb[:, c * CCHUNK:(c + 1) * CCHUNK], mult)
                nc.scalar.activation(
                    h2fm[:, c * CCHUNK:(c + 1) * CCHUNK], t[:, :], relu,
                    bias=b2_sb[:, :])
            for j in range(NT):
                zp3 = ps_z.tile([T, DL[0]], f32, tag="zp")
                nc.tensor.matmul(zp3[:, :DL[2]],
                                 h2fm[:, j * T:(j + 1) * T],
                                 w3_sb[:, :], start=True, stop=True)
                zb3 = zbpool.tile([T, DL[2]], bf16, tag="zb3")
                nc.vector.tensor_scalar(
                    zb3[:, :], zp3[:, :DL[2]], dis_sb[:, j:j + 1], None, mult)
                z_store(2, j, zb3)
                if j == HT_A - 1:
                    issue_ag(2, 0)
                if j == NT - 1:
                    issue_ag(2, 1)

            # ---- L3 scatter ----
            scat_state["s"] = 0
            scatter_steps(2, S_TILES, DL[2])

            # ---- L3 post + output transpose ----
            for c in range(NCH):
                t = tpool.tile([T, CCHUNK], f32, tag="fmpost")
                nc.vector.tensor_tensor(
                    t[:DL[2], :], fm[c][:DL[2], :],
                    disrow_sb[:DL[2], c * CCHUNK:(c + 1) * CCHUNK], mult)
                if apply_b3:
                    nc.scalar.activation(
                        outfm[:DL[2], c * CCHUNK:(c + 1) * CCHUNK],
                        t[:DL[2], :], fcopy, bias=b3_sb[:DL[2], :])
                else:
                    nc.scalar.activation(
                        outfm[:DL[2], c * CCHUNK:(c + 1) * CCHUNK],
                        t[:DL[2], :], fcopy)
            for j in range(NT):
                tpf = ps_z.tile([T, DL[0]], f32, tag="zp")
                nc.tensor.matmul(tpf[:, :DL[2]],
                                 outfm[:DL[2], j * T:(j + 1) * T],
                                 identf_sb[:DL[2], :DL[2]],
                                 is_transpose=True)
                ot = htpool.tile([T, DL[2]], f32, tag="ot")
                nc.scalar.activation(ot[:, :], tpf[:, :DL[2]], fcopy)
                nc.sync.dma_start(out=out_v[:, j, :], in_=ot[:, :])

    nc.compile()
    return nc


# ----------------------------------------------------------------------------
# Host-side preprocessing
# ----------------------------------------------------------------------------
def _balanced_node_order(deg):
    """Deal nodes (sorted by degree desc) snake-wise over core*tile buckets."""
    n_buckets = N_CORES * NT
    slots_total = n_buckets * T
    by_deg = np.argsort(-deg, kind="stable")
    node_order = -np.ones(slots_total, np.int64)
    new_pos = np.zeros(N_NODES, np.int64)
    fill = np.zeros(n_buckets, np.int64)
    b = 0
    direction = 1
    for node in by_deg:
        node_order[b * T + fill[b]] = node
        new_pos[node] = b * T + fill[b]
        fill[b] += 1
        b += direction
        if b == n_buckets:
            b = n_buckets - 1
            direction = -1
        elif b < 0:
            b = 0
            direction = 1
    return node_order, new_pos


def _preprocess(edge_index):
    src = np.asarray(edge_index[0], dtype=np.int64)
    dst = np.asarray(edge_index[1], dtype=np.int64)
    deg = np.bincount(dst, minlength=N_NODES).astype(np.float64) + 1.0
    dis_full = 1.0 / np.sqrt(deg)

    node_order, new_pos = _balanced_node_order(deg)

    spos = new_pos[src]
    dpos = new_pos[dst]
    core = dpos // SHARD
    tile = (dpos % SHARD) // T
    slot = dpos % T
    shalf = ((spos % SHARD) >= HALF_A).astype(np.int64)
    srow_half = ((spos // SHARD) * np.where(shalf == 0, HALF_A, HALF_B)
                 + (spos % SHARD) - shalf * HALF_A)

    counts = np.zeros((N_CORES, 2, NT), np.int64)
    np.add.at(counts, (core, shalf, tile), 1)
    CNT = [[max(16, _ru16(counts[:, h, j].max())) for j in range(NT)]
           for h in range(2)]

    nbl = [[(CNT[h][j] + T - 1) // T for j in range(NT)] for h in range(2)]
    boff = [[0] * NT for _ in range(2)]
    ioff16 = [[0] * NT for _ in range(2)]
    ob = oi = 0
    for h in range(2):
        for j in range(NT):
            boff[h][j] = ob
            ioff16[h][j] = oi
            ob += nbl[h][j]
            oi += CNT[h][j] // 16
    totblk = ob
    idxcols = oi

    order = np.lexsort((slot, tile, shalf, core))
    core_s = core[order]
    tile_s = tile[order]
    slot_s = slot[order]
    half_s = shalf[order]
    row_s = srow_half[order]

    grp = (core_s * 2 + half_s) * NT + tile_s
    grp_start = np.zeros(N_CORES * 2 * NT + 1, np.int64)
    np.add.at(grp_start, grp + 1, 1)
    grp_start = np.cumsum(grp_start)
    rank = np.arange(len(grp)) - grp_start[grp]

    boff_np = np.array(boff)        # [2, NT] blocks
    ioff_np = np.array(ioff16)      # [2, NT] 16-col units
    # flat index position of each edge within its core's idx array
    epos = ioff_np[half_s, tile_s] * 16 + rank
    # sel position: block and lane within group
    blk = boff_np[half_s, tile_s] + rank // T
    lane = rank % T

    idx_cores, sel_cores, dis_cores, disrow_cores = [], [], [], []
    for c in range(N_CORES):
        m = core_s == c
        flat = np.zeros(idxcols * 16, np.int16)
        flat[epos[m]] = row_s[m].astype(np.int16)
        wrapped = np.tile(flat.reshape(idxcols, 16).T, (8, 1))
        idx_cores.append(np.ascontiguousarray(wrapped.astype(np.int16)))

        selc = np.zeros((totblk, T, T), np.uint8)
        selc[blk[m], lane[m], slot_s[m]] = 1
        sel_cores.append(np.ascontiguousarray(
            selc.transpose(1, 0, 2).reshape(T, totblk * T)).astype(FP8))

        slots = node_order[c * SHARD:(c + 1) * SHARD]
        dis_c = np.where(slots >= 0, dis_full[np.maximum(slots, 0)], 0.0)
        dis_cores.append(np.ascontiguousarray(
            dis_c.reshape(NT, T).T).astype(np.float32))
        disrow_cores.append(np.ascontiguousarray(
            np.tile(dis_c[None, :], (T, 1))).astype(np.float32))

    # ---- S matrices (with self-loops), rows in [half-a | half-b] order ----
    loop_pos = new_pos[node_order[node_order >= 0]]  # all real slots
    s_all = np.concatenate([spos, loop_pos])
    d_all = np.concatenate([dpos, loop_pos])
    sh_all = ((s_all % SHARD) >= HALF_A).astype(np.int64)
    srow_glob = np.where(
        sh_all == 0,
        (s_all // SHARD) * HALF_A + (s_all % SHARD),
        N_CORES * HALF_A + (s_all // SHARD) * HALF_B
        + (s_all % SHARD) - HALF_A)
    dcore_all = d_all // SHARD
    dloc_all = d_all % SHARD
    smat_cores = []
    for c in range(N_CORES):
        m = dcore_all == c
        S = np.zeros((NTAB, SHARD), np.uint8)
        np.add.at(S, (srow_glob[m], dloc_all[m]), 1)
        smat_cores.append(S.astype(FP8))

    return (idx_cores, sel_cores, dis_cores, disrow_cores, smat_cores,
            CNT, node_order)


def _make_in_maps(x, W1, b1, W2, b2, W3, b3, edge_index):
    (idx_cores, sel_cores, dis_cores, disrow_cores, smat_cores,
     CNT, node_order) = _preprocess(edge_index)

    x = np.asarray(x, np.float32)
    w1b = np.asarray(W1, np.float32).astype(BF16)
    w2b = np.asarray(W2, np.float32).astype(BF16)
    w3b = np.asarray(W3, np.float32).astype(BF16)
    b1f = np.asarray(b1, np.float32)
    b2f = np.asarray(b2, np.float32)
    b3f = np.asarray(b3, np.float32)
    brep1 = np.tile(b1f, (T, 1))
    b2col = np.zeros((T, 1), np.float32)
    b2col[:DL[1], 0] = b2f
    b3col = np.zeros((T, 1), np.float32)
    b3col[:DL[2], 0] = b3f
    identb = np.eye(T, dtype=BF16)
    identf = np.eye(T, dtype=np.float32)
    apply_b1 = bool(np.any(b1f))
    apply_b3 = bool(np.any(b3f))

    in_maps = []
    for c in range(N_CORES):
        slots = node_order[c * SHARD:(c + 1) * SHARD]
        xs = np.where((slots >= 0)[:, None], x[np.maximum(slots, 0)], 0.0)
        in_maps.append({
            "xt": np.ascontiguousarray(xs.T.astype(np.float32)).astype(BF16),
            "w1": w1b, "w2": w2b, "w3": w3b,
            "brep1": brep1, "b2col": b2col, "b3col": b3col,
            "dis": dis_cores[c], "disrow": disrow_cores[c],
            "identb": identb, "identf": identf,
            "idx": idx_cores[c], "sel": sel_cores[c],
            "smat": smat_cores[c],
        })
    return in_maps, CNT, node_order, apply_b1, apply_b3


_NC_CACHE = {}


def kernel_with_results(x, W1, b1, W2, b2, W3, b3, edge_index, trace=False):
    in_maps, CNT, node_order, apply_b1, apply_b3 = _make_in_maps(
        x, W1, b1, W2, b2, W3, b3, edge_index)
    key = (tuple(CNT[0]), tuple(CNT[1]), apply_b1, apply_b3)
    if key not in _NC_CACHE:
        _NC_CACHE[key] = _build_nc(CNT, apply_b1, apply_b3)
    nc = _NC_CACHE[key]
    res = run_bass_kernel_spmd(
        nc, in_maps, core_ids=list(range(N_CORES)), trace=trace)
    rows = np.concatenate(
        [np.asarray(res.results[c]["out"]) for c in range(N_CORES)], axis=0)
    full = np.zeros((N_NODES, rows.shape[1]), np.float32)
    real = node_order >= 0
    full[node_order[real]] = rows[real]
    return full, res


def kernel(x, W1, b1, W2, b2, W3, b3, edge_index):
    full, _ = kernel_with_results(x, W1, b1, W2, b2, W3, b3, edge_index)
    return full


# revision 15
# speedup vs baseline: 1.8384x; 1.1627x over previous
"""GCN (3-layer, PyG GCNConv-style) forward pass on 8 Trainium2 NeuronCores.

Architecture v3 (gather L1 + hot/cold split PE-scatter L2/L3):
  - Nodes are assigned to tiles by OUT-degree bands (band k -> tile k on
    every core; within a band, snake-dealt by IN-degree across cores), so
    tile index correlates with out-degree.  Tiles >= JCUT hold the
    lowest-out-degree nodes ("cold"), the rest are "hot".
  - Z_l = dis * (H @ Wl) computed per core, AllGathered (bf16, halves a/b
    for overlap) so each core holds the full 20480-row table in HBM.  The
    layer-3 table is padded to 128 columns (gather needs 256B rows).
  - Layer 1 aggregation: SWDGE dma_gather + fp8-selector matmuls (gather
    costs ~7-9 ns/row of GpSimd regardless of width, so it handles the
    widest layer).  Self-loops enter via an identity matmul on the local
    Z tile; gather calls use exact per-group counts on 4 SWDGE queues.
  - Layers 2/3, hot source tiles (104 of 160): PE-scatter.  Z_s is the
    stationary operand; a 0/1 adjacency slice S_s [128 src x 2560 dst]
    (fp8, self-loops included) streams as the moving operand, accumulating
    a feature-major PSUM [d x 2560] in five 512-col bank chunks.
  - Layers 2/3, cold source tiles: their edges are dma_gathered (GpSimd is
    otherwise idle during the scatter phases) and folded into the same
    feature-major PSUM via per-dst-tile selector matmuls with the gathered
    block as the stationary operand.
  - Post-ops run feature-major (dis is a replicated row tensor, bias is
    per-partition), the next GEMM consumes H_fm directly as lhsT (no
    transposes), and the final output is PE-transposed back to node-major.
  - Scatter steps for half-a source tiles interleave into the tail of the
    layer-1 phase-B loop so the PE works while GpSimd still gathers.
"""

import sys

import numpy as np

sys.path.insert(0, "/opt/trn_rl_repo")

import ml_dtypes  # noqa: E402

import concourse.bass as bass  # noqa: E402
import concourse.bacc as bacc  # noqa: E402
import concourse.mybir as mybir  # noqa: E402
from concourse.bass_utils import run_bass_kernel_spmd  # noqa: E402
from concourse.library_config import mlp as _mlp_lib  # noqa: E402
from concourse.tile import TileContext  # noqa: E402
from concourse.tile_rust import add_dep_helper  # noqa: E402

BF16 = ml_dtypes.bfloat16
FP8 = ml_dtypes.float8_e4m3

# ----------------------------------------------------------------------------
# Problem configuration (hardcoded for nn_Encoder_17386027614431)
# ----------------------------------------------------------------------------
N_NODES = 20000
N_CORES = 8
T = 128
NT = 20                  # dst tiles per core
SHARD = NT * T           # 2560
NTAB = N_CORES * SHARD   # 20480 table rows
D0 = 256
DL = [256, 128, 64]      # per-layer output dims
HT_A = 4                 # tiles in AllGather half a
JCUT = 13                # tiles >= JCUT are cold: L2/L3 edges via gather
HALF_A = HT_A * T        # 512
HALF_B = SHARD - HALF_A  # 2048
HSZ = [HALF_A, HALF_B]
SA_TILES = N_CORES * HT_A          # 32 src tiles in half-a table
S_TILES = NTAB // T                # 160
HOTB = JCUT - HT_A                 # hot half-b tiles per core (9)
NCOLD = NT - JCUT                  # cold tiles per core (7)
DPAD = [256, 128, 128]             # agout row widths (L3 padded to 128)
CCHUNK = 512                       # psum bank columns (f32)
NCH = SHARD // CCHUNK              # 5 feature-major column chunks


def _ru16(x):
    return (int(x) + 15) // 16 * 16


def _offsets(cnt2d):
    """Block / idx-column offsets for a [2][NT] or [NT] count table."""
    flat = [c for row in cnt2d for c in row]
    nbl = [(c + T - 1) // T for c in flat]
    boff, ioff = [], []
    ob = oi = 0
    for c, nb in zip(flat, nbl):
        boff.append(ob)
        ioff.append(oi)
        ob += nb
        oi += c // 16
    return nbl, boff, ioff, ob, oi


def _build_nc(CNT, CNT2, apply_b1, apply_b3):
    """CNT: [2][NT] L1 gather counts; CNT2: [NT] cold L2/L3 gather counts
    (multiples of 16), identical on every core."""
    f32 = mybir.dt.float32
    bf16 = mybir.dt.bfloat16
    fp8 = mybir.dt.float8e4
    i16 = mybir.dt.int16
    mult = mybir.AluOpType.mult
    add = mybir.AluOpType.add
    relu = mybir.ActivationFunctionType.Relu
    fcopy = mybir.ActivationFunctionType.Copy

    nbl_f, boff_f, ioff_f, totblk, idxcols = _offsets(CNT)
    nbl = [nbl_f[:NT], nbl_f[NT:]]
    boff = [boff_f[:NT], boff_f[NT:]]
    ioff = [ioff_f[:NT], ioff_f[NT:]]
    nbl2, boff2, ioff2, totblk2, idxcols2 = _offsets([CNT2])
    maxnb = max(max(nbl[0]), max(nbl[1]), max(nbl2))

    nc = bacc.Bacc("TRN2", num_devices=N_CORES, num_swdge_queues=4)

    # ---- kernel I/O ----
    xt = nc.dram_tensor("xt", [D0, SHARD], bf16, kind="ExternalInput")
    w1 = nc.dram_tensor("w1", [D0, DL[0]], bf16, kind="ExternalInput")
    w2 = nc.dram_tensor("w2", [DL[0], DL[1]], bf16, kind="ExternalInput")
    w3 = nc.dram_tensor("w3", [DL[1], DL[2]], bf16, kind="ExternalInput")
    brep1 = nc.dram_tensor("brep1", [T, DL[0]], f32, kind="ExternalInput")
    b2col = nc.dram_tensor("b2col", [T, 1], f32, kind="ExternalInput")
    b3col = nc.dram_tensor("b3col", [T, 1], f32, kind="ExternalInput")
    dis = nc.dram_tensor("dis", [T, NT], f32, kind="ExternalInput")
    disrow = nc.dram_tensor("disrow", [T, SHARD], f32, kind="ExternalInput")
    identb = nc.dram_tensor("identb", [T, T], bf16, kind="ExternalInput")
    identf = nc.dram_tensor("identf", [T, T], f32, kind="ExternalInput")
    idx = nc.dram_tensor("idx", [T, idxcols], i16, kind="ExternalInput")
    sel = nc.dram_tensor("sel", [T, totblk * T], fp8, kind="ExternalInput")
    idx2 = nc.dram_tensor("idx2", [T, idxcols2], i16, kind="ExternalInput")
    sel2 = nc.dram_tensor("sel2", [T, totblk2 * T], fp8,
                          kind="ExternalInput")
    smat = nc.dram_tensor("smat", [NTAB, SHARD], fp8, kind="ExternalInput")
    out = nc.dram_tensor("out", [SHARD, DL[2]], f32, kind="ExternalOutput")

    # ---- internal DRAM for collectives ----
    agin = [[nc.dram_tensor(f"agin{l}_{h}", [HSZ[h], DPAD[l]], bf16)
             for h in range(2)] for l in range(3)]
    agout = [[nc.dram_tensor(f"agout{l}_{h}", [N_CORES * HSZ[h], DPAD[l]],
                             bf16, addr_space="Shared")
              for h in range(2)] for l in range(3)]
    rg = [list(range(N_CORES))]

    with TileContext(nc) as tc:
        nc.gpsimd.load_library(_mlp_lib)

        with (
            tc.tile_pool(name="const", bufs=1) as cpool,
            tc.tile_pool(name="sb", bufs=4) as sbpool,        # S stream
            tc.tile_pool(name="zsb", bufs=2) as zspool,       # Z stationary
            tc.tile_pool(name="selp", bufs=3) as selpool,
            tc.tile_pool(name="hp", bufs=2) as hpool,
            tc.tile_pool(name="htp", bufs=3) as htpool,
            tc.tile_pool(name="tmp", bufs=3) as tpool,
            tc.tile_pool(name="zbp", bufs=3) as zbpool,
            tc.tile_pool(name="ps_z", bufs=1, space="PSUM") as ps_z,
            tc.tile_pool(name="ps_agg", bufs=1, space="PSUM") as ps_agg,
            tc.tile_pool(name="ps_t", bufs=1, space="PSUM") as ps_t,
            tc.tile_pool(name="ps_fm", bufs=1, space="PSUM") as ps_fm,
        ):
            # ---- constants ----
            def load_const(dram_h, shape, dtype):
                t = cpool.tile(shape, dtype, tag=f"c_{dram_h.name}")
                nc.sync.dma_start(out=t[:, :], in_=dram_h.ap())
                return t

            def load_const_chunked(dram_h, inner, dtype):
                cs = dram_h.shape[0] // T
                t = cpool.tile([T, cs * inner], dtype, tag=f"c_{dram_h.name}")
                nc.sync.dma_start(
                    out=t.rearrange("p (c n) -> p c n", c=cs),
                    in_=dram_h.ap().rearrange("(c p) n -> p c n", p=T),
                )
                return t

            idx_sb = load_const(idx, [T, idxcols], i16)
            idx2_sb = load_const(idx2, [T, idxcols2], i16)
            xt_sb = load_const_chunked(xt, SHARD, bf16)
            w1_sb = load_const_chunked(w1, DL[0], bf16)
            w2_sb = load_const_chunked(w2, DL[1], bf16)
            w3_sb = load_const(w3, [DL[1], DL[2]], bf16)
            brep1_sb = load_const(brep1, [T, DL[0]], f32)
            b2_sb = load_const(b2col, [T, 1], f32)
            b3_sb = load_const(b3col, [T, 1], f32)
            dis_sb = load_const(dis, [T, NT], f32)
            disrow_sb = load_const(disrow, [T, SHARD], f32)
            identb_sb = load_const(identb, [T, T], bf16)
            identf_sb = load_const(identf, [T, T], f32)

            # persistent buffers
            gbuf = [cpool.tile([T, maxnb * DL[0]], bf16, tag=f"g{i}",
                               name=f"gbuf{i}") for i in range(3)]
            for g in gbuf:
                nc.gpsimd.memset(g[:, :], 0.0)
            zb1 = [cpool.tile([T, DL[0]], bf16, tag=f"zb1_{j}",
                              name=f"zb1_{j}") for j in range(NT)]
            acc = cpool.tile([T, NT * DL[0]], f32, tag="acc")
            h2fm = cpool.tile([T, SHARD], bf16, tag="h2fm")
            outfm = cpool.tile([T, SHARD], f32, tag="outfm")
            fm = [ps_fm.tile([T, CCHUNK], f32, tag=f"fm{c}",
                             name=f"fm{c}") for c in range(NCH)]

            agin_v = [[agin[l][h].ap().rearrange("(n p) d -> p n d", p=T)
                       for h in range(2)] for l in range(3)]
            agouta_v = [agout[l][0].ap().rearrange("(n p) d -> p n d", p=T)
                        for l in range(3)]
            agoutb_v = [agout[l][1].ap().rearrange("(n p) d -> p n d", p=T)
                        for l in range(3)]
            smat_v = smat.ap().rearrange("(s p) d -> p s d", p=T)
            out_v = out.ap().rearrange("(n p) d -> p n d", p=T)

            ag_insts = [[None, None] for _ in range(3)]
            agin_dmas = [[[], []] for _ in range(3)]

            def z_store(l, j, zb):
                h = 0 if j < HT_A else 1
                d = nc.sync.dma_start(
                    out=agin_v[l][h][:, j - h * HT_A, :], in_=zb[:, :])
                agin_dmas[l][h].append(d)

            def issue_ag(l, h):
                cc = nc.gpsimd.collective_compute(
                    "AllGather",
                    mybir.AluOpType.bypass,
                    replica_groups=rg,
                    ins=[agin[l][h].ap().opt()],
                    outs=[agout[l][h].ap().opt()],
                )
                for d in agin_dmas[l][h]:
                    add_dep_helper(cc.ins, d.ins, reason=f"ag{l}.{h}")
                ag_insts[l][h] = cc

            # ================= Layer 1: Z1 = dis * (x @ W1) ================
            for j in range(NT):
                zp = ps_z.tile([T, DL[0]], f32, tag="zp")
                for c in range(2):
                    nc.tensor.matmul(
                        zp[:, :],
                        xt_sb[:, c * SHARD + j * T: c * SHARD + (j + 1) * T],
                        w1_sb[:, c * DL[0]:(c + 1) * DL[0]],
                        start=(c == 0), stop=(c == 1),
                    )
                nc.vector.tensor_scalar(
                    zb1[j][:, :], zp[:, :], dis_sb[:, j:j + 1], None, mult)
                z_store(0, j, zb1[j])
                if j == HT_A - 1:
                    issue_ag(0, 0)
            issue_ag(0, 1)

            # ---- gather helper (L1) ----
            gq = [0]

            def gather_group(h, j, gslot):
                cnt = CNT[h][j]
                nb = nbl[h][j]
                gt3 = gbuf[gslot][:, :nb * DL[0]].rearrange(
                    "p (n d) -> p n d", d=DL[0])
                g = nc.gpsimd.dma_gather(
                    gt3,
                    agout[0][h].ap(),
                    idx_sb[:, ioff[h][j]:ioff[h][j] + cnt // 16],
                    cnt, cnt, DL[0],
                    single_packet=False,
                    queue_num=gq[0] % 4,
                )
                gq[0] += 1
                add_dep_helper(g.ins, ag_insts[0][h].ins, reason="g after ag")
                st = selpool.tile([T, maxnb * T], fp8, tag="sel")
                nc.sync.dma_start(
                    out=st[:, :nb * T],
                    in_=sel[:, boff[h][j] * T:(boff[h][j] + nb) * T])
                return gt3, st, nb

            # ---- cold-gather for L2/L3 (src tiles >= JCUT, all half-b) ----
            nb2max = max(nbl2)
            cbuf = [cpool.tile([T, nb2max * DPAD[1]], bf16, tag=f"cb{j}",
                               name=f"cbuf{j}") for j in range(NT)]
            for cb in cbuf:
                nc.gpsimd.memset(cb[:, :], 0.0)

            def cold_gathers(l):
                """Gather every dst tile's cold edges from agout[l][1]."""
                for j in range(NT):
                    cnt = CNT2[j]
                    gt3 = cbuf[j][:, :nbl2[j] * DPAD[l]].rearrange(
                        "p (n d) -> p n d", d=DPAD[l])
                    g = nc.gpsimd.dma_gather(
                        gt3,
                        agout[l][1].ap(),
                        idx2_sb[:, ioff2[j]:ioff2[j] + cnt // 16],
                        cnt, cnt, DPAD[l],
                        single_packet=False,
                        queue_num=gq[0] % 4,
                    )
                    gq[0] += 1
                    add_dep_helper(g.ins, ag_insts[l][1].ins, reason="cg ag")

            def cold_matmuls(l):
                """Fold gathered cold edges into the fm psum; close each
                chunk's accumulation group on its last writer."""
                d_el = DL[l]
                for j in range(NT):
                    nb = nbl2[j]
                    gt3 = cbuf[j][:, :nb * DPAD[l]].rearrange(
                        "p (n d) -> p n d", d=DPAD[l])
                    st = selpool.tile([T, maxnb * T], fp8, tag="sel")
                    nc.sync.dma_start(
                        out=st[:, :nb * T],
                        in_=sel2[:, boff2[j] * T:(boff2[j] + nb) * T])
                    c, r = j // 4, (j % 4) * T
                    for b in range(nb):
                        nc.tensor.matmul(
                            fm[c][:d_el, r:r + T],
                            gt3[:, b, :d_el],
                            st[:, b * T:(b + 1) * T],
                            start=False,
                            stop=(j % 4 == 3 and b == nb - 1),
                            skip_group_check=True)

            # ---- L2/L3 hot-scatter machinery ----
            # hot src tiles: all of half-a (tiles 0..HT_A-1 per core, rows
            # [c*HALF_A,(c+1)*HALF_A)), plus half-b tiles HT_A..JCUT-1 per
            # core (first HOTB tiles of each core's half-b stripe).
            scat = {"pos": 0, "zsb": None}
            n_hot = SA_TILES + N_CORES * HOTB

            def scatter_steps(l, n, limit):
                d_el = DL[l]
                dp = DPAD[l]
                while n > 0 and scat["pos"] < limit:
                    pos = scat["pos"]
                    if pos < SA_TILES:
                        bsz = 8
                        if pos % bsz == 0:
                            zsb = zspool.tile([T, bsz * dp], bf16,
                                              tag=f"zs{l}")
                            d = nc.sync.dma_start(
                                out=zsb.rearrange("p (n d) -> p n d", d=dp),
                                in_=agouta_v[l][:, pos:pos + bsz, :])
                            add_dep_helper(d.ins, ag_insts[l][0].ins,
                                           reason="zs ag")
                            scat["zsb"] = zsb
                        k = pos % bsz
                    else:
                        bsz = HOTB
                        p = pos - SA_TILES
                        if p % bsz == 0:
                            core = p // bsz
                            row0 = core * (HALF_B // T)
                            zsb = zspool.tile([T, bsz * dp], bf16,
                                              tag=f"zs{l}")
                            d = nc.sync.dma_start(
                                out=zsb.rearrange("p (n d) -> p n d", d=dp),
                                in_=agoutb_v[l][:, row0:row0 + bsz, :])
                            add_dep_helper(d.ins, ag_insts[l][1].ins,
                                           reason="zs ag")
                            scat["zsb"] = zsb
                        k = p % bsz
                    stile = sbpool.tile([T, SHARD], fp8, tag="smat")
                    if pos < SA_TILES:
                        srow = pos
                    else:
                        p = pos - SA_TILES
                        srow = (SA_TILES + (p // HOTB) * (HALF_B // T)
                                + (p % HOTB))
                    nc.sync.dma_start(out=stile[:, :], in_=smat_v[:, srow, :])
                    zsb = scat["zsb"]
                    for c in range(NCH):
                        nc.tensor.matmul(
                            fm[c][:d_el, :],
                            zsb[:, k * dp:k * dp + d_el],
                            stile[:, c * CCHUNK:(c + 1) * CCHUNK],
                            start=(pos == 0), stop=False,
                            skip_group_check=True)
                    scat["pos"] = pos + 1
                    n -= 1

            def finish_agg(l):
                """Remaining half-a steps, then cold gathers (GpSimd works
                while the PE runs the hot half-b steps), then cold matmuls
                close the accumulation groups."""
                scatter_steps(l, SA_TILES, SA_TILES)
                cold_gathers(l)
                scatter_steps(l, n_hot, n_hot)
                cold_matmuls(l)

            # ================= L1 phase A (half 0) =========================
            for j in range(NT):
                gt3, st, nb = gather_group(0, j, j % 3)
                ps = ps_agg.tile([T, DL[0]], f32, tag="agg")
                nc.tensor.matmul(ps[:, :], identb_sb[:, :], zb1[j][:, :],
                                 start=True, stop=False)
                for b in range(nb):
                    nc.tensor.matmul(
                        ps[:, :], st[:, b * T:(b + 1) * T], gt3[:, b, :],
                        start=False, stop=(b == nb - 1))
                nc.scalar.activation(
                    acc[:, j * DL[0]:(j + 1) * DL[0]], ps[:, :], fcopy)

            # ================= L1 phase B + post + Z2 + AG2 ================
            for j in range(NT):
                gt3, st, nb = gather_group(1, j, j % 3)
                ps = ps_agg.tile([T, DL[0]], f32, tag="agg")
                for b in range(nb):
                    nc.tensor.matmul(
                        ps[:, :], st[:, b * T:(b + 1) * T], gt3[:, b, :],
                        start=(b == 0), stop=(b == nb - 1))
                u = tpool.tile([T, DL[0]], f32, tag="post")
                nc.vector.tensor_tensor(
                    u[:, :], ps[:, :], acc[:, j * DL[0]:(j + 1) * DL[0]], add)
                if apply_b1:
                    u2 = tpool.tile([T, DL[0]], f32, tag="post")
                    nc.vector.tensor_tensor(u2[:, :], u[:, :], brep1_sb[:, :],
                                            add)
                    u = u2
                h1 = hpool.tile([T, DL[0]], bf16, tag="h1")
                nc.scalar.activation(h1[:, :], u[:, :], relu,
                                     scale=dis_sb[:, j:j + 1])
                zp2 = ps_z.tile([T, DL[0]], f32, tag="zp")
                for c in range(2):
                    tp = ps_t.tile([T, T], bf16, tag="tp")
                    nc.tensor.matmul(tp[:, :], h1[:, c * T:(c + 1) * T],
                                     identb_sb[:, :], is_transpose=True)
                    htc = htpool.tile([T, T], bf16, tag="ht")
                    nc.scalar.activation(htc[:, :], tp[:, :], fcopy)
                    nc.tensor.matmul(
                        zp2[:, :DL[1]], htc[:, :],
                        w2_sb[:, c * DL[1]:(c + 1) * DL[1]],
                        start=(c == 0), stop=(c == 1))
                zb2 = zbpool.tile([T, DL[1]], bf16, tag="zb2")
                nc.vector.tensor_scalar(
                    zb2[:, :], zp2[:, :DL[1]], dis_sb[:, j:j + 1], None, mult)
                z_store(1, j, zb2)
                if j == HT_A - 1:
                    issue_ag(1, 0)
                if j == NT - 1:
                    issue_ag(1, 1)
                if j >= HT_A:
                    scatter_steps(1, 2, SA_TILES)

            # ---- L2 aggregation remainder (hot scatter + cold gathers) ----
            finish_agg(1)

            # ---- L2 post (feature-major) + Z3 + AG3 ----
            for c in range(NCH):
                t = tpool.tile([T, CCHUNK], f32, tag="fmpost")
                nc.vector.tensor_tensor(
                    t[:, :], fm[c][:, :],
                    disrow_sb[:, c * CCHUNK:(c + 1) * CCHUNK], mult)
                nc.scalar.activation(
                    h2fm[:, c * CCHUNK:(c + 1) * CCHUNK], t[:, :], relu,
                    bias=b2_sb[:, :])
            for j in range(NT):
                zp3 = ps_z.tile([T, DL[0]], f32, tag="zp")
                nc.tensor.matmul(zp3[:, :DL[2]],
                                 h2fm[:, j * T:(j + 1) * T],
                                 w3_sb[:, :], start=True, stop=True)
                # pad cols DL[2]:128 of zb3 carry garbage; no consumer ever
                # reads past column 63 of the layer-3 table.
                zb3 = zbpool.tile([T, DPAD[2]], bf16, tag="zb3")
                nc.vector.tensor_scalar(
                    zb3[:, :DL[2]], zp3[:, :DL[2]], dis_sb[:, j:j + 1],
                    None, mult)
                z_store(2, j, zb3)
                if j == HT_A - 1:
                    issue_ag(2, 0)
                if j == NT - 1:
                    issue_ag(2, 1)

            # ---- L3 aggregation ----
            scat["pos"] = 0
            finish_agg(2)

            # ---- L3 post + output transpose ----
            for c in range(NCH):
                t = tpool.tile([T, CCHUNK], f32, tag="fmpost")
                nc.vector.tensor_tensor(
                    t[:DL[2], :], fm[c][:DL[2], :],
                    disrow_sb[:DL[2], c * CCHUNK:(c + 1) * CCHUNK], mult)
                if apply_b3:
                    nc.scalar.activation(
                        outfm[:DL[2], c * CCHUNK:(c + 1) * CCHUNK],
                        t[:DL[2], :], fcopy, bias=b3_sb[:DL[2], :])
                else:
                    nc.scalar.activation(
                        outfm[:DL[2], c * CCHUNK:(c + 1) * CCHUNK],
                        t[:DL[2], :], fcopy)
            for j in range(NT):
                tpf = ps_z.tile([T, DL[0]], f32, tag="zp")
                nc.tensor.matmul(tpf[:, :DL[2]],
                                 outfm[:DL[2], j * T:(j + 1) * T],
                                 identf_sb[:DL[2], :DL[2]],
                                 is_transpose=True)
                ot = htpool.tile([T, DL[2]], f32, tag="ot")
                nc.scalar.activation(ot[:, :], tpf[:, :DL[2]], fcopy)
                nc.sync.dma_start(out=out_v[:, j, :], in_=ot[:, :])

    nc.compile()
    return nc


# ----------------------------------------------------------------------------
# Host-side preprocessing
# ----------------------------------------------------------------------------
def _band_node_order(outdeg, indeg):
    """Band k (by out-degree rank) -> tile k on every core; within a band,
    snake-deal by in-degree across the 8 cores' buckets."""
    by_out = np.argsort(-outdeg, kind="stable")  # includes only real nodes
    node_order = -np.ones(NTAB, np.int64)
    new_pos = np.zeros(N_NODES, np.int64)
    band_sz = N_CORES * T
    for k in range(NT):
        band = by_out[k * band_sz:(k + 1) * band_sz]
        band = band[np.argsort(-indeg[band], kind="stable")]
        fill = np.zeros(N_CORES, np.int64)
        b = 0
        direction = 1
        for node in band:
            pos = b * SHARD + k * T + fill[b]
            node_order[pos] = node
            new_pos[node] = pos
            fill[b] += 1
            b += direction
            if b == N_CORES:
                b = N_CORES - 1
                direction = -1
            elif b < 0:
                b = 0
                direction = 1
    return node_order, new_pos


def _group_pack(core_s, grp, ngrp, row_s, slot_s, CNT_flat, ioff_flat,
                boff_flat):
    """Pack edges (sorted by (core, grp)) into idx/sel arrays per core."""
    grp_start = np.zeros(N_CORES * ngrp + 1, np.int64)
    np.add.at(grp_start, core_s * ngrp + grp + 1, 1)
    grp_start = np.cumsum(grp_start)
    rank = np.arange(len(grp)) - grp_start[core_s * ngrp + grp]
    cnt_np = np.array(CNT_flat)
    ioff_np = np.array(ioff_flat)
    boff_np = np.array(boff_flat)
    epos = ioff_np[grp] * 16 + rank
    blk = boff_np[grp] + rank // T
    lane = rank % T
    idxcols = int(ioff_np[-1] + cnt_np[-1] // 16)
    totblk = int(boff_np[-1] + (cnt_np[-1] + T - 1) // T)
    idx_cores, sel_cores = [], []
    for c in range(N_CORES):
        m = core_s == c
        flat = np.zeros(idxcols * 16, np.int16)
        flat[epos[m]] = row_s[m].astype(np.int16)
        wrapped = np.tile(flat.reshape(idxcols, 16).T, (8, 1))
        idx_cores.append(np.ascontiguousarray(wrapped.astype(np.int16)))
        selc = np.zeros((totblk, T, T), np.uint8)
        selc[blk[m], lane[m], slot_s[m]] = 1
        sel_cores.append(np.ascontiguousarray(
            selc.transpose(1, 0, 2).reshape(T, totblk * T)).astype(FP8))
    return idx_cores, sel_cores


def _preprocess(edge_index):
    src = np.asarray(edge_index[0], dtype=np.int64)
    dst = np.asarray(edge_index[1], dtype=np.int64)
    indeg = np.bincount(dst, minlength=N_NODES).astype(np.float64) + 1.0
    outdeg = np.bincount(src, minlength=N_NODES).astype(np.float64)
    dis_full = 1.0 / np.sqrt(indeg)

    node_order, new_pos = _band_node_order(outdeg, indeg)

    spos = new_pos[src]
    dpos = new_pos[dst]
    core = dpos // SHARD
    tile = (dpos % SHARD) // T
    slot = dpos % T
    shalf = ((spos % SHARD) >= HALF_A).astype(np.int64)
    srow_half = ((spos // SHARD) * np.where(shalf == 0, HALF_A, HALF_B)
                 + (spos % SHARD) - shalf * HALF_A)

    # ---- L1 groups: (src half, dst tile) ----
    counts = np.zeros((N_CORES, 2, NT), np.int64)
    np.add.at(counts, (core, shalf, tile), 1)
    CNT = [[max(16, _ru16(counts[:, h, j].max())) for j in range(NT)]
           for h in range(2)]
    CNT_flat = [c for row in CNT for c in row]
    _, boff_f, ioff_f, _, _ = _offsets(CNT)

    order = np.lexsort((slot, tile, shalf, core))
    g1 = shalf[order] * NT + tile[order]
    idx_cores, sel_cores = _group_pack(
        core[order], g1, 2 * NT, srow_half[order], slot[order],
        CNT_flat, ioff_f, boff_f)

    # ---- cold edges (src tile >= JCUT; all in half b) for L2/L3 ----
    loop_pos = new_pos[node_order[node_order >= 0]]
    s_all = np.concatenate([spos, loop_pos])
    d_all = np.concatenate([dpos, loop_pos])
    stile_all = (s_all % SHARD) // T
    cold_m = stile_all >= JCUT
    sc = s_all[cold_m]
    dc = d_all[cold_m]
    ccore = dc // SHARD
    ctile = (dc % SHARD) // T
    cslot = dc % T
    crow = (sc // SHARD) * HALF_B + (sc % SHARD) - HALF_A
    counts2 = np.zeros((N_CORES, NT), np.int64)
    np.add.at(counts2, (ccore, ctile), 1)
    CNT2 = [max(16, _ru16(counts2[:, j].max())) for j in range(NT)]
    _, boff2_f, ioff2_f, _, _ = _offsets([CNT2])
    order2 = np.lexsort((cslot, ctile, ccore))
    idx2_cores, sel2_cores = _group_pack(
        ccore[order2], ctile[order2], NT, crow[order2], cslot[order2],
        CNT2, ioff2_f, boff2_f)

    # ---- S matrices: hot srcs only (tile < JCUT), self-loops included ----
    hot_m = ~cold_m
    sh = s_all[hot_m]
    dh = d_all[hot_m]
    sh_half = ((sh % SHARD) >= HALF_A).astype(np.int64)
    srow_glob = np.where(
        sh_half == 0,
        (sh // SHARD) * HALF_A + (sh % SHARD),
        N_CORES * HALF_A + (sh // SHARD) * HALF_B + (sh % SHARD) - HALF_A)
    dcore_h = dh // SHARD
    dloc_h = dh % SHARD
    smat_cores = []
    for c in range(N_CORES):
        m = dcore_h == c
        S = np.zeros((NTAB, SHARD), np.uint8)
        np.add.at(S, (srow_glob[m], dloc_h[m]), 1)
        smat_cores.append(S.astype(FP8))

    dis_cores, disrow_cores = [], []
    for c in range(N_CORES):
        slots = node_order[c * SHARD:(c + 1) * SHARD]
        dis_c = np.where(slots >= 0, dis_full[np.maximum(slots, 0)], 0.0)
        dis_cores.append(np.ascontiguousarray(
            dis_c.reshape(NT, T).T).astype(np.float32))
        disrow_cores.append(np.ascontiguousarray(
            np.tile(dis_c[None, :], (T, 1))).astype(np.float32))

    return (idx_cores, sel_cores, idx2_cores, sel2_cores, dis_cores,
            disrow_cores, smat_cores, CNT, CNT2, node_order)


def _make_in_maps(x, W1, b1, W2, b2, W3, b3, edge_index):
    (idx_cores, sel_cores, idx2_cores, sel2_cores, dis_cores, disrow_cores,
     smat_cores, CNT, CNT2, node_order) = _preprocess(edge_index)

    x = np.asarray(x, np.float32)
    w1b = np.asarray(W1, np.float32).astype(BF16)
    w2b = np.asarray(W2, np.float32).astype(BF16)
    w3b = np.asarray(W3, np.float32).astype(BF16)
    b1f = np.asarray(b1, np.float32)
    b2f = np.asarray(b2, np.float32)
    b3f = np.asarray(b3, np.float32)
    brep1 = np.tile(b1f, (T, 1))
    b2col = np.zeros((T, 1), np.float32)
    b2col[:DL[1], 0] = b2f
    b3col = np.zeros((T, 1), np.float32)
    b3col[:DL[2], 0] = b3f
    identb = np.eye(T, dtype=BF16)
    identf = np.eye(T, dtype=np.float32)
    apply_b1 = bool(np.any(b1f))
    apply_b3 = bool(np.any(b3f))

    in_maps = []
    for c in range(N_CORES):
        slots = node_order[c * SHARD:(c + 1) * SHARD]
        xs = np.where((slots >= 0)[:, None], x[np.maximum(slots, 0)], 0.0)
        in_maps.append({
            "xt": np.ascontiguousarray(xs.T.astype(np.float32)).astype(BF16),
            "w1": w1b, "w2": w2b, "w3": w3b,
            "brep1": brep1, "b2col": b2col, "b3col": b3col,
            "dis": dis_cores[c], "disrow": disrow_cores[c],
            "identb": identb, "identf": identf,
            "idx": idx_cores[c], "sel": sel_cores[c],
            "idx2": idx2_cores[c], "sel2": sel2_cores[c],
            "smat": smat_cores[c],
        })
    return in_maps, CNT, CNT2, node_order, apply_b1, apply_b3


_NC_CACHE = {}


def kernel_with_results(x, W1, b1, W2, b2, W3, b3, edge_index, trace=False):
    in_maps, CNT, CNT2, node_order, apply_b1, apply_b3 = _make_in_maps(
        x, W1, b1, W2, b2, W3, b3, edge_index)
    key = (tuple(CNT[0]), tuple(CNT[1]), tuple(CNT2), apply_b1, apply_b3)
    if key not in _NC_CACHE:
        _NC_CACHE[key] = _build_nc(CNT, CNT2, apply_b1, apply_b3)
    nc = _NC_CACHE[key]
    res = run_bass_kernel_spmd(
        nc, in_maps, core_ids=list(range(N_CORES)), trace=trace)
    rows = np.concatenate(
        [np.asarray(res.results[c]["out"]) for c in range(N_CORES)], axis=0)
    full = np.zeros((N_NODES, rows.shape[1]), np.float32)
    real = node_order >= 0
    full[node_order[real]] = rows[real]
    return full, res


def kernel(x, W1, b1, W2, b2, W3, b3, edge_index):
    full, _ = kernel_with_results(x, W1, b1, W2, b2, W3, b3, edge_index)
    return full


# revision 17
# speedup vs baseline: 2.0393x; 1.1093x over previous
"""GCN (3-layer, PyG GCNConv-style) forward pass on 8 Trainium2 NeuronCores.

Architecture v3 (gather L1 + hot/cold split PE-scatter L2/L3):
  - Nodes are assigned to tiles by OUT-degree bands (band k -> tile k on
    every core; within a band, snake-dealt by IN-degree across cores), so
    tile index correlates with out-degree.  Tiles >= JCUT hold the
    lowest-out-degree nodes ("cold"), the rest are "hot".
  - Z_l = dis * (H @ Wl) computed per core, AllGathered (bf16, halves a/b
    for overlap) so each core holds the full 20480-row table in HBM.  The
    layer-3 table is padded to 128 columns (gather needs 256B rows).
  - Layer 1 aggregation: SWDGE dma_gather + fp8-selector matmuls (gather
    costs ~7-9 ns/row of GpSimd regardless of width, so it handles the
    widest layer).  Self-loops enter via an identity matmul on the local
    Z tile; gather calls use exact per-group counts on 4 SWDGE queues.
  - Layers 2/3, hot source tiles (104 of 160): PE-scatter.  Z_s is the
    stationary operand; a 0/1 adjacency slice S_s [128 src x 2560 dst]
    (fp8, self-loops included) streams as the moving operand, accumulating
    a feature-major PSUM [d x 2560] in five 512-col bank chunks.
  - Layers 2/3, cold source tiles: their edges are dma_gathered (GpSimd is
    otherwise idle during the scatter phases) and folded into the same
    feature-major PSUM via per-dst-tile selector matmuls with the gathered
    block as the stationary operand.
  - Post-ops run feature-major (dis is a replicated row tensor, bias is
    per-partition), the next GEMM consumes H_fm directly as lhsT (no
    transposes), and the final output is PE-transposed back to node-major.
  - Scatter steps for half-a source tiles interleave into the tail of the
    layer-1 phase-B loop so the PE works while GpSimd still gathers.
"""

import sys

import numpy as np

sys.path.insert(0, "/opt/trn_rl_repo")

import ml_dtypes  # noqa: E402

import concourse.bass as bass  # noqa: E402
import concourse.bacc as bacc  # noqa: E402
import concourse.mybir as mybir  # noqa: E402
from concourse.bass_utils import run_bass_kernel_spmd  # noqa: E402
from concourse.library_config import mlp as _mlp_lib  # noqa: E402
from concourse.tile import TileContext  # noqa: E402
from concourse.tile_rust import add_dep_helper  # noqa: E402

BF16 = ml_dtypes.bfloat16
FP8 = ml_dtypes.float8_e4m3

# ----------------------------------------------------------------------------
# Problem configuration (hardcoded for nn_Encoder_17386027614431)
# ----------------------------------------------------------------------------
N_NODES = 20000
N_CORES = 8
T = 128
NT = 20                  # dst tiles per core
SHARD = NT * T           # 2560
NTAB = N_CORES * SHARD   # 20480 table rows
D0 = 256
DL = [256, 128, 64]      # per-layer output dims
HT_A = 4                 # tiles in AllGather half a
JCUT = 11                # tiles >= JCUT are cold: L2/L3 edges via gather
HALF_A = HT_A * T        # 512
HALF_B = SHARD - HALF_A  # 2048
HSZ = [HALF_A, HALF_B]
SA_TILES = N_CORES * HT_A          # 32 src tiles in half-a table
S_TILES = NTAB // T                # 160
HOTB = JCUT - HT_A                 # hot half-b tiles per core (9)
NCOLD = NT - JCUT                  # cold tiles per core (7)
DPAD = [256, 128, 128]             # agout row widths (L3 padded to 128)
CCHUNK = 512                       # psum bank columns (f32)
NCH = SHARD // CCHUNK              # 5 feature-major column chunks


def _ru16(x):
    return (int(x) + 15) // 16 * 16


def _offsets(cnt2d):
    """Block / idx-column offsets for a [2][NT] or [NT] count table."""
    flat = [c for row in cnt2d for c in row]
    nbl = [(c + T - 1) // T for c in flat]
    boff, ioff = [], []
    ob = oi = 0
    for c, nb in zip(flat, nbl):
        boff.append(ob)
        ioff.append(oi)
        ob += nb
        oi += c // 16
    return nbl, boff, ioff, ob, oi


def _build_nc(CNT, CNT2, apply_b1, apply_b3):
    """CNT: [2][NT] L1 gather counts; CNT2: [NT] cold L2/L3 gather counts
    (multiples of 16), identical on every core."""
    f32 = mybir.dt.float32
    bf16 = mybir.dt.bfloat16
    fp8 = mybir.dt.float8e4
    i16 = mybir.dt.int16
    mult = mybir.AluOpType.mult
    add = mybir.AluOpType.add
    relu = mybir.ActivationFunctionType.Relu
    fcopy = mybir.ActivationFunctionType.Copy

    nbl_f, boff_f, ioff_f, totblk, idxcols = _offsets(CNT)
    nbl = [nbl_f[:NT], nbl_f[NT:]]
    boff = [boff_f[:NT], boff_f[NT:]]
    ioff = [ioff_f[:NT], ioff_f[NT:]]
    nbl2, boff2, ioff2, totblk2, idxcols2 = _offsets([CNT2])
    maxnb = max(max(nbl[0]), max(nbl[1]), max(nbl2))

    nc = bacc.Bacc("TRN2", num_devices=N_CORES, num_swdge_queues=4)

    # ---- kernel I/O ----
    xt = nc.dram_tensor("xt", [D0, SHARD], bf16, kind="ExternalInput")
    w1 = nc.dram_tensor("w1", [D0, DL[0]], bf16, kind="ExternalInput")
    w2 = nc.dram_tensor("w2", [DL[0], DL[1]], bf16, kind="ExternalInput")
    w3 = nc.dram_tensor("w3", [DL[1], DL[2]], bf16, kind="ExternalInput")
    brep1 = nc.dram_tensor("brep1", [T, DL[0]], f32, kind="ExternalInput")
    b2col = nc.dram_tensor("b2col", [T, 1], f32, kind="ExternalInput")
    b3col = nc.dram_tensor("b3col", [T, 1], f32, kind="ExternalInput")
    dis = nc.dram_tensor("dis", [T, NT], f32, kind="ExternalInput")
    disrow = nc.dram_tensor("disrow", [T, SHARD], f32, kind="ExternalInput")
    identb = nc.dram_tensor("identb", [T, T], bf16, kind="ExternalInput")
    identf = nc.dram_tensor("identf", [T, T], f32, kind="ExternalInput")
    idx = nc.dram_tensor("idx", [T, idxcols], i16, kind="ExternalInput")
    sel = nc.dram_tensor("sel", [T, totblk * T], fp8, kind="ExternalInput")
    idx2 = nc.dram_tensor("idx2", [T, idxcols2], i16, kind="ExternalInput")
    sel2 = nc.dram_tensor("sel2", [T, totblk2 * T], fp8,
                          kind="ExternalInput")
    smat = nc.dram_tensor("smat", [NTAB, SHARD], fp8, kind="ExternalInput")
    out = nc.dram_tensor("out", [SHARD, DL[2]], f32, kind="ExternalOutput")

    # ---- internal DRAM for collectives ----
    agin = [[nc.dram_tensor(f"agin{l}_{h}", [HSZ[h], DPAD[l]], bf16)
             for h in range(2)] for l in range(3)]
    agout = [[nc.dram_tensor(f"agout{l}_{h}", [N_CORES * HSZ[h], DPAD[l]],
                             bf16, addr_space="Shared")
              for h in range(2)] for l in range(3)]
    rg = [list(range(N_CORES))]

    with TileContext(nc) as tc:
        nc.gpsimd.load_library(_mlp_lib)

        with (
            tc.tile_pool(name="const", bufs=1) as cpool,
            tc.tile_pool(name="sb", bufs=6) as sbpool,        # S stream
            tc.tile_pool(name="zsb", bufs=2) as zspool,       # Z stationary
            tc.tile_pool(name="selp", bufs=3) as selpool,
            tc.tile_pool(name="hp", bufs=2) as hpool,
            tc.tile_pool(name="htp", bufs=3) as htpool,
            tc.tile_pool(name="tmp", bufs=3) as tpool,
            tc.tile_pool(name="zbp", bufs=3) as zbpool,
            tc.tile_pool(name="ps_z", bufs=1, space="PSUM") as ps_z,
            tc.tile_pool(name="ps_agg", bufs=1, space="PSUM") as ps_agg,
            tc.tile_pool(name="ps_t", bufs=1, space="PSUM") as ps_t,
            tc.tile_pool(name="ps_fm", bufs=1, space="PSUM") as ps_fm,
        ):
            # ---- constants ----
            def load_const(dram_h, shape, dtype):
                t = cpool.tile(shape, dtype, tag=f"c_{dram_h.name}")
                nc.sync.dma_start(out=t[:, :], in_=dram_h.ap())
                return t

            def load_const_chunked(dram_h, inner, dtype):
                cs = dram_h.shape[0] // T
                t = cpool.tile([T, cs * inner], dtype, tag=f"c_{dram_h.name}")
                nc.sync.dma_start(
                    out=t.rearrange("p (c n) -> p c n", c=cs),
                    in_=dram_h.ap().rearrange("(c p) n -> p c n", p=T),
                )
                return t

            xt_sb = load_const_chunked(xt, SHARD, bf16)
            w1_sb = load_const_chunked(w1, DL[0], bf16)
            dis_sb = load_const(dis, [T, NT], f32)
            identb_sb = load_const(identb, [T, T], bf16)
            idx_sb = load_const(idx, [T, idxcols], i16)
            idx2_sb = load_const(idx2, [T, idxcols2], i16)
            w2_sb = load_const_chunked(w2, DL[1], bf16)
            w3_sb = load_const(w3, [DL[1], DL[2]], bf16)
            brep1_sb = load_const(brep1, [T, DL[0]], f32)
            b2_sb = load_const(b2col, [T, 1], f32)
            b3_sb = load_const(b3col, [T, 1], f32)
            disrow_sb = load_const(disrow, [T, SHARD], f32)
            identf_sb = load_const(identf, [T, T], f32)

            # persistent buffers
            gbuf = [cpool.tile([T, maxnb * DL[0]], bf16, tag=f"g{i}",
                               name=f"gbuf{i}") for i in range(3)]
            for g in gbuf:
                nc.gpsimd.memset(g[:, :], 0.0)
            zb1 = [cpool.tile([T, DL[0]], bf16, tag=f"zb1_{j}",
                              name=f"zb1_{j}") for j in range(NT)]
            acc = cpool.tile([T, NT * DL[0]], f32, tag="acc")
            h2fm = cpool.tile([T, SHARD], bf16, tag="h2fm")
            outfm = cpool.tile([T, SHARD], f32, tag="outfm")
            fm = [ps_fm.tile([T, CCHUNK], f32, tag=f"fm{c}",
                             name=f"fm{c}") for c in range(NCH)]

            agin_v = [[agin[l][h].ap().rearrange("(n p) d -> p n d", p=T)
                       for h in range(2)] for l in range(3)]
            agouta_v = [agout[l][0].ap().rearrange("(n p) d -> p n d", p=T)
                        for l in range(3)]
            agoutb_v = [agout[l][1].ap().rearrange("(n p) d -> p n d", p=T)
                        for l in range(3)]
            smat_v = smat.ap().rearrange("(s p) d -> p s d", p=T)
            out_v = out.ap().rearrange("(n p) d -> p n d", p=T)

            ag_insts = [[None, None] for _ in range(3)]
            agin_dmas = [[[], []] for _ in range(3)]

            def z_store(l, j, zb):
                h = 0 if j < HT_A else 1
                d = nc.sync.dma_start(
                    out=agin_v[l][h][:, j - h * HT_A, :], in_=zb[:, :])
                agin_dmas[l][h].append(d)

            def issue_ag(l, h):
                cc = nc.gpsimd.collective_compute(
                    "AllGather",
                    mybir.AluOpType.bypass,
                    replica_groups=rg,
                    ins=[agin[l][h].ap().opt()],
                    outs=[agout[l][h].ap().opt()],
                )
                for d in agin_dmas[l][h]:
                    add_dep_helper(cc.ins, d.ins, reason=f"ag{l}.{h}")
                ag_insts[l][h] = cc

            # ================= Layer 1: Z1 = dis * (x @ W1) ================
            for j in range(NT):
                zp = ps_z.tile([T, DL[0]], f32, tag="zp")
                for c in range(2):
                    nc.tensor.matmul(
                        zp[:, :],
                        xt_sb[:, c * SHARD + j * T: c * SHARD + (j + 1) * T],
                        w1_sb[:, c * DL[0]:(c + 1) * DL[0]],
                        start=(c == 0), stop=(c == 1),
                    )
                nc.vector.tensor_scalar(
                    zb1[j][:, :], zp[:, :], dis_sb[:, j:j + 1], None, mult)
                z_store(0, j, zb1[j])
                if j == HT_A - 1:
                    issue_ag(0, 0)
            issue_ag(0, 1)

            # ---- gather helper (L1) ----
            gq = [0]

            def gather_group(h, j, gslot):
                cnt = CNT[h][j]
                nb = nbl[h][j]
                gt3 = gbuf[gslot][:, :nb * DL[0]].rearrange(
                    "p (n d) -> p n d", d=DL[0])
                g = nc.gpsimd.dma_gather(
                    gt3,
                    agout[0][h].ap(),
                    idx_sb[:, ioff[h][j]:ioff[h][j] + cnt // 16],
                    cnt, cnt, DL[0],
                    single_packet=False,
                    queue_num=gq[0] % 4,
                )
                gq[0] += 1
                add_dep_helper(g.ins, ag_insts[0][h].ins, reason="g after ag")
                st = selpool.tile([T, maxnb * T], fp8, tag="sel")
                nc.sync.dma_start(
                    out=st[:, :nb * T],
                    in_=sel[:, boff[h][j] * T:(boff[h][j] + nb) * T])
                return gt3, st, nb

            # ---- cold-gather for L2/L3 (src tiles >= JCUT, all half-b) ----
            nb2max = max(nbl2)
            cbuf = [cpool.tile([T, nb2max * DPAD[1]], bf16, tag=f"cb{j}",
                               name=f"cbuf{j}") for j in range(NT)]
            for cb in cbuf:
                nc.gpsimd.memset(cb[:, :], 0.0)

            def cold_gathers(l):
                """Gather every dst tile's cold edges from agout[l][1]."""
                for j in range(NT):
                    cnt = CNT2[j]
                    gt3 = cbuf[j][:, :nbl2[j] * DPAD[l]].rearrange(
                        "p (n d) -> p n d", d=DPAD[l])
                    g = nc.gpsimd.dma_gather(
                        gt3,
                        agout[l][1].ap(),
                        idx2_sb[:, ioff2[j]:ioff2[j] + cnt // 16],
                        cnt, cnt, DPAD[l],
                        single_packet=False,
                        queue_num=gq[0] % 4,
                    )
                    gq[0] += 1
                    add_dep_helper(g.ins, ag_insts[l][1].ins, reason="cg ag")

            def cold_matmuls(l):
                """Fold gathered cold edges into the fm psum; close each
                chunk's accumulation group on its last writer."""
                d_el = DL[l]
                for j in range(NT):
                    nb = nbl2[j]
                    gt3 = cbuf[j][:, :nb * DPAD[l]].rearrange(
                        "p (n d) -> p n d", d=DPAD[l])
                    st = selpool.tile([T, maxnb * T], fp8, tag="sel")
                    nc.sync.dma_start(
                        out=st[:, :nb * T],
                        in_=sel2[:, boff2[j] * T:(boff2[j] + nb) * T])
                    c, r = j // 4, (j % 4) * T
                    for b in range(nb):
                        nc.tensor.matmul(
                            fm[c][:d_el, r:r + T],
                            gt3[:, b, :d_el],
                            st[:, b * T:(b + 1) * T],
                            start=False,
                            stop=(j % 4 == 3 and b == nb - 1),
                            skip_group_check=True)

            # ---- L2/L3 hot-scatter machinery ----
            # hot src tiles: all of half-a (tiles 0..HT_A-1 per core, rows
            # [c*HALF_A,(c+1)*HALF_A)), plus half-b tiles HT_A..JCUT-1 per
            # core (first HOTB tiles of each core's half-b stripe).
            scat = {"pos": 0, "zsb": None}
            n_hot = SA_TILES + N_CORES * HOTB

            def scatter_steps(l, n, limit):
                d_el = DL[l]
                dp = DPAD[l]
                while n > 0 and scat["pos"] < limit:
                    pos = scat["pos"]
                    if pos < SA_TILES:
                        bsz = 8
                        if pos % bsz == 0:
                            zsb = zspool.tile([T, bsz * dp], bf16,
                                              tag=f"zs{l}")
                            d = nc.scalar.dma_start(
                                out=zsb.rearrange("p (n d) -> p n d", d=dp),
                                in_=agouta_v[l][:, pos:pos + bsz, :])
                            add_dep_helper(d.ins, ag_insts[l][0].ins,
                                           reason="zs ag")
                            scat["zsb"] = zsb
                        k = pos % bsz
                    else:
                        bsz = HOTB
                        p = pos - SA_TILES
                        if p % bsz == 0:
                            core = p // bsz
                            row0 = core * (HALF_B // T)
                            zsb = zspool.tile([T, bsz * dp], bf16,
                                              tag=f"zs{l}")
                            d = nc.scalar.dma_start(
                                out=zsb.rearrange("p (n d) -> p n d", d=dp),
                                in_=agoutb_v[l][:, row0:row0 + bsz, :])
                            add_dep_helper(d.ins, ag_insts[l][1].ins,
                                           reason="zs ag")
                            scat["zsb"] = zsb
                        k = p % bsz
                    stile = sbpool.tile([T, SHARD], fp8, tag="smat")
                    if pos < SA_TILES:
                        srow = pos
                    else:
                        p = pos - SA_TILES
                        srow = (SA_TILES + (p // HOTB) * (HALF_B // T)
                                + (p % HOTB))
                    nc.scalar.dma_start(out=stile[:, :],
                                        in_=smat_v[:, srow, :])
                    zsb = scat["zsb"]
                    for c in range(NCH):
                        nc.tensor.matmul(
                            fm[c][:d_el, :],
                            zsb[:, k * dp:k * dp + d_el],
                            stile[:, c * CCHUNK:(c + 1) * CCHUNK],
                            start=(pos == 0), stop=False,
                            skip_group_check=True)
                    scat["pos"] = pos + 1
                    n -= 1

            def finish_agg(l):
                """Remaining half-a steps, then cold gathers (GpSimd works
                while the PE runs the hot half-b steps), then cold matmuls
                close the accumulation groups."""
                scatter_steps(l, SA_TILES, SA_TILES)
                cold_gathers(l)
                scatter_steps(l, n_hot, n_hot)
                cold_matmuls(l)

            # ================= L1 phase A (half 0) =========================
            for j in range(NT):
                gt3, st, nb = gather_group(0, j, j % 3)
                ps = ps_agg.tile([T, DL[0]], f32, tag="agg")
                nc.tensor.matmul(ps[:, :], identb_sb[:, :], zb1[j][:, :],
                                 start=True, stop=False)
                for b in range(nb):
                    nc.tensor.matmul(
                        ps[:, :], st[:, b * T:(b + 1) * T], gt3[:, b, :],
                        start=False, stop=(b == nb - 1))
                nc.scalar.activation(
                    acc[:, j * DL[0]:(j + 1) * DL[0]], ps[:, :], fcopy)

            # ================= L1 phase B + post + Z2 + AG2 ================
            for j in range(NT):
                gt3, st, nb = gather_group(1, j, j % 3)
                ps = ps_agg.tile([T, DL[0]], f32, tag="agg")
                for b in range(nb):
                    nc.tensor.matmul(
                        ps[:, :], st[:, b * T:(b + 1) * T], gt3[:, b, :],
                        start=(b == 0), stop=(b == nb - 1))
                u = tpool.tile([T, DL[0]], f32, tag="post")
                nc.vector.tensor_tensor(
                    u[:, :], ps[:, :], acc[:, j * DL[0]:(j + 1) * DL[0]], add)
                if apply_b1:
                    u2 = tpool.tile([T, DL[0]], f32, tag="post")
                    nc.vector.tensor_tensor(u2[:, :], u[:, :], brep1_sb[:, :],
                                            add)
                    u = u2
                h1 = hpool.tile([T, DL[0]], bf16, tag="h1")
                nc.scalar.activation(h1[:, :], u[:, :], relu,
                                     scale=dis_sb[:, j:j + 1])
                zp2 = ps_z.tile([T, DL[0]], f32, tag="zp")
                for c in range(2):
                    tp = ps_t.tile([T, T], bf16, tag="tp")
                    nc.tensor.matmul(tp[:, :], h1[:, c * T:(c + 1) * T],
                                     identb_sb[:, :], is_transpose=True)
                    htc = htpool.tile([T, T], bf16, tag="ht")
                    nc.scalar.activation(htc[:, :], tp[:, :], fcopy)
                    nc.tensor.matmul(
                        zp2[:, :DL[1]], htc[:, :],
                        w2_sb[:, c * DL[1]:(c + 1) * DL[1]],
                        start=(c == 0), stop=(c == 1))
                zb2 = zbpool.tile([T, DL[1]], bf16, tag="zb2")
                nc.vector.tensor_scalar(
                    zb2[:, :], zp2[:, :DL[1]], dis_sb[:, j:j + 1], None, mult)
                z_store(1, j, zb2)
                if j == HT_A - 1:
                    issue_ag(1, 0)
                if j == NT - 1:
                    issue_ag(1, 1)
                if j >= HT_A:
                    scatter_steps(1, 2, SA_TILES)

            # ---- L2 aggregation remainder (hot scatter + cold gathers) ----
            finish_agg(1)

            # ---- L2 post (feature-major) + Z3 + AG3 ----
            for c in range(NCH):
                t = tpool.tile([T, CCHUNK], f32, tag="fmpost")
                nc.vector.tensor_tensor(
                    t[:, :], fm[c][:, :],
                    disrow_sb[:, c * CCHUNK:(c + 1) * CCHUNK], mult)
                nc.scalar.activation(
                    h2fm[:, c * CCHUNK:(c + 1) * CCHUNK], t[:, :], relu,
                    bias=b2_sb[:, :])
            for j in range(NT):
                zp3 = ps_z.tile([T, DL[0]], f32, tag="zp")
                nc.tensor.matmul(zp3[:, :DL[2]],
                                 h2fm[:, j * T:(j + 1) * T],
                                 w3_sb[:, :], start=True, stop=True)
                # pad cols DL[2]:128 of zb3 carry garbage; no consumer ever
                # reads past column 63 of the layer-3 table.
                zb3 = zbpool.tile([T, DPAD[2]], bf16, tag="zb3")
                nc.vector.tensor_scalar(
                    zb3[:, :DL[2]], zp3[:, :DL[2]], dis_sb[:, j:j + 1],
                    None, mult)
                z_store(2, j, zb3)
                if j == HT_A - 1:
                    issue_ag(2, 0)
                if j == NT - 1:
                    issue_ag(2, 1)

            # ---- L3 aggregation ----
            scat["pos"] = 0
            finish_agg(2)

            # ---- L3 post + output transpose ----
            for c in range(NCH):
                t = tpool.tile([T, CCHUNK], f32, tag="fmpost")
                nc.vector.tensor_tensor(
                    t[:DL[2], :], fm[c][:DL[2], :],
                    disrow_sb[:DL[2], c * CCHUNK:(c + 1) * CCHUNK], mult)
                if apply_b3:
                    nc.scalar.activation(
                        outfm[:DL[2], c * CCHUNK:(c + 1) * CCHUNK],
                        t[:DL[2], :], fcopy, bias=b3_sb[:DL[2], :])
                else:
                    nc.scalar.activation(
                        outfm[:DL[2], c * CCHUNK:(c + 1) * CCHUNK],
                        t[:DL[2], :], fcopy)
            for j in range(NT):
                tpf = ps_z.tile([T, DL[0]], f32, tag="zp")
                nc.tensor.matmul(tpf[:, :DL[2]],
                                 outfm[:DL[2], j * T:(j + 1) * T],
                                 identf_sb[:DL[2], :DL[2]],
                                 is_transpose=True)
                ot = htpool.tile([T, DL[2]], f32, tag="ot")
                nc.scalar.activation(ot[:, :], tpf[:, :DL[2]], fcopy)
                nc.sync.dma_start(out=out_v[:, j, :], in_=ot[:, :])

    nc.compile()
    return nc


# ----------------------------------------------------------------------------
# Host-side preprocessing
# ----------------------------------------------------------------------------
def _band_node_order(outdeg, indeg):
    """Band k (by out-degree rank) -> tile k on every core; within a band,
    snake-deal by in-degree across the 8 cores' buckets."""
    by_out = np.argsort(-outdeg, kind="stable")  # includes only real nodes
    node_order = -np.ones(NTAB, np.int64)
    new_pos = np.zeros(N_NODES, np.int64)
    band_sz = N_CORES * T
    for k in range(NT):
        band = by_out[k * band_sz:(k + 1) * band_sz]
        band = band[np.argsort(-indeg[band], kind="stable")]
        fill = np.zeros(N_CORES, np.int64)
        b = 0
        direction = 1
        for node in band:
            pos = b * SHARD + k * T + fill[b]
            node_order[pos] = node
            new_pos[node] = pos
            fill[b] += 1
            b += direction
            if b == N_CORES:
                b = N_CORES - 1
                direction = -1
            elif b < 0:
                b = 0
                direction = 1
    return node_order, new_pos


def _group_pack(core_s, grp, ngrp, row_s, slot_s, CNT_flat, ioff_flat,
                boff_flat):
    """Pack edges (sorted by (core, grp)) into idx/sel arrays per core."""
    grp_start = np.zeros(N_CORES * ngrp + 1, np.int64)
    np.add.at(grp_start, core_s * ngrp + grp + 1, 1)
    grp_start = np.cumsum(grp_start)
    rank = np.arange(len(grp)) - grp_start[core_s * ngrp + grp]
    cnt_np = np.array(CNT_flat)
    ioff_np = np.array(ioff_flat)
    boff_np = np.array(boff_flat)
    epos = ioff_np[grp] * 16 + rank
    blk = boff_np[grp] + rank // T
    lane = rank % T
    idxcols = int(ioff_np[-1] + cnt_np[-1] // 16)
    totblk = int(boff_np[-1] + (cnt_np[-1] + T - 1) // T)
    idx_cores, sel_cores = [], []
    for c in range(N_CORES):
        m = core_s == c
        flat = np.zeros(idxcols * 16, np.int16)
        flat[epos[m]] = row_s[m].astype(np.int16)
        wrapped = np.tile(flat.reshape(idxcols, 16).T, (8, 1))
        idx_cores.append(np.ascontiguousarray(wrapped.astype(np.int16)))
        selc = np.zeros((totblk, T, T), np.uint8)
        selc[blk[m], lane[m], slot_s[m]] = 1
        sel_cores.append(np.ascontiguousarray(
            selc.transpose(1, 0, 2).reshape(T, totblk * T)).astype(FP8))
    return idx_cores, sel_cores


def _preprocess(edge_index):
    src = np.asarray(edge_index[0], dtype=np.int64)
    dst = np.asarray(edge_index[1], dtype=np.int64)
    indeg = np.bincount(dst, minlength=N_NODES).astype(np.float64) + 1.0
    outdeg = np.bincount(src, minlength=N_NODES).astype(np.float64)
    dis_full = 1.0 / np.sqrt(indeg)

    node_order, new_pos = _band_node_order(outdeg, indeg)

    spos = new_pos[src]
    dpos = new_pos[dst]
    core = dpos // SHARD
    tile = (dpos % SHARD) // T
    slot = dpos % T
    shalf = ((spos % SHARD) >= HALF_A).astype(np.int64)
    srow_half = ((spos // SHARD) * np.where(shalf == 0, HALF_A, HALF_B)
                 + (spos % SHARD) - shalf * HALF_A)

    # ---- L1 groups: (src half, dst tile) ----
    counts = np.zeros((N_CORES, 2, NT), np.int64)
    np.add.at(counts, (core, shalf, tile), 1)
    CNT = [[max(16, _ru16(counts[:, h, j].max())) for j in range(NT)]
           for h in range(2)]
    CNT_flat = [c for row in CNT for c in row]
    _, boff_f, ioff_f, _, _ = _offsets(CNT)

    order = np.lexsort((slot, tile, shalf, core))
    g1 = shalf[order] * NT + tile[order]
    idx_cores, sel_cores = _group_pack(
        core[order], g1, 2 * NT, srow_half[order], slot[order],
        CNT_flat, ioff_f, boff_f)

    # ---- cold edges (src tile >= JCUT; all in half b) for L2/L3 ----
    loop_pos = new_pos[node_order[node_order >= 0]]
    s_all = np.concatenate([spos, loop_pos])
    d_all = np.concatenate([dpos, loop_pos])
    stile_all = (s_all % SHARD) // T
    cold_m = stile_all >= JCUT
    sc = s_all[cold_m]
    dc = d_all[cold_m]
    ccore = dc // SHARD
    ctile = (dc % SHARD) // T
    cslot = dc % T
    crow = (sc // SHARD) * HALF_B + (sc % SHARD) - HALF_A
    counts2 = np.zeros((N_CORES, NT), np.int64)
    np.add.at(counts2, (ccore, ctile), 1)
    CNT2 = [max(16, _ru16(counts2[:, j].max())) for j in range(NT)]
    _, boff2_f, ioff2_f, _, _ = _offsets([CNT2])
    order2 = np.lexsort((cslot, ctile, ccore))
    idx2_cores, sel2_cores = _group_pack(
        ccore[order2], ctile[order2], NT, crow[order2], cslot[order2],
        CNT2, ioff2_f, boff2_f)

    # ---- S matrices: hot srcs only (tile < JCUT), self-loops included ----
    hot_m = ~cold_m
    sh = s_all[hot_m]
    dh = d_all[hot_m]
    sh_half = ((sh % SHARD) >= HALF_A).astype(np.int64)
    srow_glob = np.where(
        sh_half == 0,
        (sh // SHARD) * HALF_A + (sh % SHARD),
        N_CORES * HALF_A + (sh // SHARD) * HALF_B + (sh % SHARD) - HALF_A)
    dcore_h = dh // SHARD
    dloc_h = dh % SHARD
    smat_cores = []
    for c in range(N_CORES):
        m = dcore_h == c
        S = np.zeros((NTAB, SHARD), np.uint8)
        np.add.at(S, (srow_glob[m], dloc_h[m]), 1)
        smat_cores.append(S.astype(FP8))

    dis_cores, disrow_cores = [], []
    for c in range(N_CORES):
        slots = node_order[c * SHARD:(c + 1) * SHARD]
        dis_c = np.where(slots >= 0, dis_full[np.maximum(slots, 0)], 0.0)
        dis_cores.append(np.ascontiguousarray(
            dis_c.reshape(NT, T).T).astype(np.float32))
        disrow_cores.append(np.ascontiguousarray(
            np.tile(dis_c[None, :], (T, 1))).astype(np.float32))

    return (idx_cores, sel_cores, idx2_cores, sel2_cores, dis_cores,
            disrow_cores, smat_cores, CNT, CNT2, node_order)


def _make_in_maps(x, W1, b1, W2, b2, W3, b3, edge_index):
    (idx_cores, sel_cores, idx2_cores, sel2_cores, dis_cores, disrow_cores,
     smat_cores, CNT, CNT2, node_order) = _preprocess(edge_index)

    x = np.asarray(x, np.float32)
    w1b = np.asarray(W1, np.float32).astype(BF16)
    w2b = np.asarray(W2, np.float32).astype(BF16)
    w3b = np.asarray(W3, np.float32).astype(BF16)
    b1f = np.asarray(b1, np.float32)
    b2f = np.asarray(b2, np.float32)
    b3f = np.asarray(b3, np.float32)
    brep1 = np.tile(b1f, (T, 1))
    b2col = np.zeros((T, 1), np.float32)
    b2col[:DL[1], 0] = b2f
    b3col = np.zeros((T, 1), np.float32)
    b3col[:DL[2], 0] = b3f
    identb = np.eye(T, dtype=BF16)
    identf = np.eye(T, dtype=np.float32)
    apply_b1 = bool(np.any(b1f))
    apply_b3 = bool(np.any(b3f))

    in_maps = []
    for c in range(N_CORES):
        slots = node_order[c * SHARD:(c + 1) * SHARD]
        xs = np.where((slots >= 0)[:, None], x[np.maximum(slots, 0)], 0.0)
        in_maps.append({
            "xt": np.ascontiguousarray(xs.T.astype(np.float32)).astype(BF16),
            "w1": w1b, "w2": w2b, "w3": w3b,
            "brep1": brep1, "b2col": b2col, "b3col": b3col,
            "dis": dis_cores[c], "disrow": disrow_cores[c],
            "identb": identb, "identf": identf,
            "idx": idx_cores[c], "sel": sel_cores[c],
            "idx2": idx2_cores[c], "sel2": sel2_cores[c],
            "smat": smat_cores[c],
        })
    return in_maps, CNT, CNT2, node_order, apply_b1, apply_b3


_NC_CACHE = {}


def kernel_with_results(x, W1, b1, W2, b2, W3, b3, edge_index, trace=False):
    in_maps, CNT, CNT2, node_order, apply_b1, apply_b3 = _make_in_maps(
        x, W1, b1, W2, b2, W3, b3, edge_index)
    key = (tuple(CNT[0]), tuple(CNT[1]), tuple(CNT2), apply_b1, apply_b3)
    if key not in _NC_CACHE:
        _NC_CACHE[key] = _build_nc(CNT, CNT2, apply_b1, apply_b3)
    nc = _NC_CACHE[key]
    res = run_bass_kernel_spmd(
        nc, in_maps, core_ids=list(range(N_CORES)), trace=trace)
    rows = np.concatenate(
        [np.asarray(res.results[c]["out"]) for c in range(N_CORES)], axis=0)
    full = np.zeros((N_NODES, rows.shape[1]), np.float32)
    real = node_order >= 0
    full[node_order[real]] = rows[real]
    return full, res


def kernel(x, W1, b1, W2, b2, W3, b3, edge_index):
    full, _ = kernel_with_results(x, W1, b1, W2, b2, W3, b3, edge_index)
    return full


# revision 18
# speedup vs baseline: 2.1375x; 1.0482x over previous
"""GCN (3-layer, PyG GCNConv-style) forward pass on 8 Trainium2 NeuronCores.

Architecture v5 (gather L1 + hot/cold split PE-scatter L2/L3, chunked AG):
  - Nodes are assigned to tiles by OUT-degree bands (band k -> tile k on
    every core; within a band, snake-dealt by IN-degree across cores), so
    tile index correlates with out-degree.  Tiles >= JCUT hold the
    lowest-out-degree nodes ("cold"), the rest are "hot".
  - Z_l = dis * (H @ Wl) computed per core and AllGathered in chunks:
    layer 1 in halves (a = tiles 0..3, b = 4..19); layers 2/3 in three
    chunks aligned with the hot/cold boundary (a = 0..3, b1 = 4..JCUT-1
    hot, b2 = JCUT..19 cold) so the hot scatter can begin as soon as b1
    lands while cold gathers wait for b2.  Layer-3 rows padded to 128
    (gather needs 256B rows).
  - Layer 1 aggregation: SWDGE dma_gather + fp8-selector matmuls (gather
    costs ~7 ns/row of GpSimd regardless of width -> it handles the widest
    layer).  Self-loops enter via an identity matmul on the local Z tile;
    gather calls use exact per-group counts on 4 SWDGE queues.
  - Layers 2/3, hot source tiles: PE-scatter.  Z_s is stationary, a 0/1
    adjacency slice S_s [128 src x 2560 dst] (fp8, self-loops included)
    streams as the moving operand (fetched two tiles per DMA on the
    Activation engine's HWDGE queue), accumulating a feature-major PSUM
    [d x 2560] in five 512-col bank chunks.
  - Layers 2/3, cold source tiles: edges dma_gathered (GpSimd idles during
    scatter otherwise) and folded into the same PSUM via per-dst-tile
    selector matmuls with the gathered block stationary.
  - The layer tail is pipelined per 512-col chunk: close chunk c (cold
    matmuls) -> feature-major post -> next-layer GEMM for its 4 node tiles
    -> AllGather chunk fires as soon as its tiles are stored.
  - Post-ops run feature-major (dis as a replicated row, bias per
    partition); the next GEMM consumes H_fm directly as lhsT; the final
    output is PE-transposed back to node-major.
"""

import sys

import numpy as np

sys.path.insert(0, "/opt/trn_rl_repo")

import ml_dtypes  # noqa: E402

import concourse.bass as bass  # noqa: E402
import concourse.bacc as bacc  # noqa: E402
import concourse.mybir as mybir  # noqa: E402
from concourse.bass_utils import run_bass_kernel_spmd  # noqa: E402
from concourse.library_config import mlp as _mlp_lib  # noqa: E402
from concourse.tile import TileContext  # noqa: E402
from concourse.tile_rust import add_dep_helper  # noqa: E402

BF16 = ml_dtypes.bfloat16
FP8 = ml_dtypes.float8_e4m3

# ----------------------------------------------------------------------------
# Problem configuration (hardcoded for nn_Encoder_17386027614431)
# ----------------------------------------------------------------------------
N_NODES = 20000
N_CORES = 8
T = 128
NT = 20                  # dst tiles per core
SHARD = NT * T           # 2560
NTAB = N_CORES * SHARD   # 20480 table rows
D0 = 256
DL = [256, 128, 64]      # per-layer output dims
HT_A = 4                 # tiles in AllGather chunk a
JCUT = 11                # tiles >= JCUT are cold: L2/L3 edges via gather
HALF_A = HT_A * T
HALF_B = SHARD - HALF_A
SA_TILES = N_CORES * HT_A          # 32 src tiles in chunk-a table
HOTB = JCUT - HT_A                 # hot b1 tiles per core (7)
NCOLD = NT - JCUT                  # cold tiles per core (9)
DPAD = [256, 128, 128]             # table row widths (L3 padded)
CCHUNK = 512                       # psum bank columns (f32)
NCH = SHARD // CCHUNK              # 5 feature-major column chunks
# AG chunk tile ranges per layer
AGCH = [
    [(0, HT_A), (HT_A, NT)],
    [(0, HT_A), (HT_A, JCUT), (JCUT, NT)],
    [(0, HT_A), (HT_A, JCUT), (JCUT, NT)],
]


def _ru16(x):
    return (int(x) + 15) // 16 * 16


def _offsets(cnt2d):
    flat = [c for row in cnt2d for c in row]
    nbl = [(c + T - 1) // T for c in flat]
    boff, ioff = [], []
    ob = oi = 0
    for c, nb in zip(flat, nbl):
        boff.append(ob)
        ioff.append(oi)
        ob += nb
        oi += c // 16
    return nbl, boff, ioff, ob, oi


def _build_nc(CNT, CNT2, apply_b1, apply_b3):
    f32 = mybir.dt.float32
    bf16 = mybir.dt.bfloat16
    fp8 = mybir.dt.float8e4
    i16 = mybir.dt.int16
    mult = mybir.AluOpType.mult
    add = mybir.AluOpType.add
    relu = mybir.ActivationFunctionType.Relu
    fcopy = mybir.ActivationFunctionType.Copy

    nbl_f, boff_f, ioff_f, totblk, idxcols = _offsets(CNT)
    nbl = [nbl_f[:NT], nbl_f[NT:]]
    boff = [boff_f[:NT], boff_f[NT:]]
    ioff = [ioff_f[:NT], ioff_f[NT:]]
    nbl2, boff2, ioff2, totblk2, idxcols2 = _offsets([CNT2])
    maxnb = max(max(nbl[0]), max(nbl[1]), max(nbl2))

    nc = bacc.Bacc("TRN2", num_devices=N_CORES, num_swdge_queues=4)

    # ---- kernel I/O ----
    xt = nc.dram_tensor("xt", [D0, SHARD], bf16, kind="ExternalInput")
    w1 = nc.dram_tensor("w1", [D0, DL[0]], bf16, kind="ExternalInput")
    w2 = nc.dram_tensor("w2", [DL[0], DL[1]], bf16, kind="ExternalInput")
    w3 = nc.dram_tensor("w3", [DL[1], DL[2]], bf16, kind="ExternalInput")
    brep1 = nc.dram_tensor("brep1", [T, DL[0]], f32, kind="ExternalInput")
    b2col = nc.dram_tensor("b2col", [T, 1], f32, kind="ExternalInput")
    b3col = nc.dram_tensor("b3col", [T, 1], f32, kind="ExternalInput")
    dis = nc.dram_tensor("dis", [T, NT], f32, kind="ExternalInput")
    disrow = nc.dram_tensor("disrow", [T, SHARD], f32, kind="ExternalInput")
    identb = nc.dram_tensor("identb", [T, T], bf16, kind="ExternalInput")
    identf = nc.dram_tensor("identf", [T, T], f32, kind="ExternalInput")
    idx = nc.dram_tensor("idx", [T, idxcols], i16, kind="ExternalInput")
    sel = nc.dram_tensor("sel", [T, totblk * T], fp8, kind="ExternalInput")
    idx2 = nc.dram_tensor("idx2", [T, idxcols2], i16, kind="ExternalInput")
    sel2 = nc.dram_tensor("sel2", [T, totblk2 * T], fp8,
                          kind="ExternalInput")
    smat = nc.dram_tensor("smat", [NTAB, SHARD], fp8, kind="ExternalInput")
    out = nc.dram_tensor("out", [SHARD, DL[2]], f32, kind="ExternalOutput")

    # ---- internal DRAM for collectives (per layer, per AG chunk) ----
    agin, agout = [], []
    for l in range(3):
        ai, ao = [], []
        for k, (j0, j1) in enumerate(AGCH[l]):
            rows = (j1 - j0) * T
            ai.append(nc.dram_tensor(f"agin{l}_{k}", [rows, DPAD[l]], bf16))
            ao.append(nc.dram_tensor(
                f"agout{l}_{k}", [N_CORES * rows, DPAD[l]], bf16,
                addr_space="Shared"))
        agin.append(ai)
        agout.append(ao)
    rg = [list(range(N_CORES))]

    with TileContext(nc) as tc:
        nc.gpsimd.load_library(_mlp_lib)

        with (
            tc.tile_pool(name="const", bufs=1) as cpool,
            tc.tile_pool(name="sb", bufs=4) as sbpool,        # S stream
            tc.tile_pool(name="zsb", bufs=2) as zspool,       # Z stationary
            tc.tile_pool(name="selp", bufs=3) as selpool,
            tc.tile_pool(name="hp", bufs=2) as hpool,
            tc.tile_pool(name="htp", bufs=3) as htpool,
            tc.tile_pool(name="tmp", bufs=3) as tpool,
            tc.tile_pool(name="zbp", bufs=3) as zbpool,
            tc.tile_pool(name="ps_z", bufs=1, space="PSUM") as ps_z,
            tc.tile_pool(name="ps_agg", bufs=1, space="PSUM") as ps_agg,
            tc.tile_pool(name="ps_t", bufs=1, space="PSUM") as ps_t,
            tc.tile_pool(name="ps_fm", bufs=1, space="PSUM") as ps_fm,
        ):
            # ---- constants (xt/w1/dis first so Z1 starts immediately) ----
            def load_const(dram_h, shape, dtype):
                t = cpool.tile(shape, dtype, tag=f"c_{dram_h.name}")
                nc.sync.dma_start(out=t[:, :], in_=dram_h.ap())
                return t

            def load_const_chunked(dram_h, inner, dtype):
                cs = dram_h.shape[0] // T
                t = cpool.tile([T, cs * inner], dtype, tag=f"c_{dram_h.name}")
                nc.sync.dma_start(
                    out=t.rearrange("p (c n) -> p c n", c=cs),
                    in_=dram_h.ap().rearrange("(c p) n -> p c n", p=T),
                )
                return t

            xt_sb = load_const_chunked(xt, SHARD, bf16)
            w1_sb = load_const_chunked(w1, DL[0], bf16)
            dis_sb = load_const(dis, [T, NT], f32)
            identb_sb = load_const(identb, [T, T], bf16)
            idx_sb = load_const(idx, [T, idxcols], i16)
            idx2_sb = load_const(idx2, [T, idxcols2], i16)
            w2_sb = load_const_chunked(w2, DL[1], bf16)
            w3_sb = load_const(w3, [DL[1], DL[2]], bf16)
            brep1_sb = load_const(brep1, [T, DL[0]], f32)
            b2_sb = load_const(b2col, [T, 1], f32)
            b3_sb = load_const(b3col, [T, 1], f32)
            disrow_sb = load_const(disrow, [T, SHARD], f32)
            identf_sb = load_const(identf, [T, T], f32)

            # persistent buffers
            gbuf = [cpool.tile([T, maxnb * DL[0]], bf16, tag=f"g{i}",
                               name=f"gbuf{i}") for i in range(3)]
            for g in gbuf:
                nc.gpsimd.memset(g[:, :], 0.0)
            zb1 = [cpool.tile([T, DL[0]], bf16, tag=f"zb1_{j}",
                              name=f"zb1_{j}") for j in range(NT)]
            acc = cpool.tile([T, NT * DL[0]], f32, tag="acc")
            h2fm = cpool.tile([T, SHARD], bf16, tag="h2fm")
            outfm = cpool.tile([T, SHARD], f32, tag="outfm")
            fm = [ps_fm.tile([T, CCHUNK], f32, tag=f"fm{c}",
                             name=f"fm{c}") for c in range(NCH)]
            nb2max = max(nbl2)
            cbuf = [cpool.tile([T, nb2max * DPAD[1]], bf16, tag=f"cb{j}",
                               name=f"cbuf{j}") for j in range(NT)]
            for cb in cbuf:
                nc.gpsimd.memset(cb[:, :], 0.0)

            agin_v = [[agin[l][k].ap().rearrange("(n p) d -> p n d", p=T)
                       for k in range(len(AGCH[l]))] for l in range(3)]
            agout_v = [[agout[l][k].ap().rearrange("(n p) d -> p n d", p=T)
                        for k in range(len(AGCH[l]))] for l in range(3)]
            smat_v = smat.ap().rearrange("(s p) d -> p s d", p=T)
            out_v = out.ap().rearrange("(n p) d -> p n d", p=T)

            ag_insts = [[None] * len(AGCH[l]) for l in range(3)]
            agin_dmas = [[[] for _ in AGCH[l]] for l in range(3)]

            def z_store(l, j, zb):
                for k, (j0, j1) in enumerate(AGCH[l]):
                    if j0 <= j < j1:
                        break
                d = nc.sync.dma_start(
                    out=agin_v[l][k][:, j - j0, :], in_=zb[:, :])
                agin_dmas[l][k].append(d)

            def issue_ag(l, k):
                cc = nc.gpsimd.collective_compute(
                    "AllGather",
                    mybir.AluOpType.bypass,
                    replica_groups=rg,
                    ins=[agin[l][k].ap().opt()],
                    outs=[agout[l][k].ap().opt()],
                )
                for d in agin_dmas[l][k]:
                    add_dep_helper(cc.ins, d.ins, reason=f"ag{l}.{k}")
                ag_insts[l][k] = cc

            # ================= Layer 1: Z1 = dis * (x @ W1) ================
            for j in range(NT):
                zp = ps_z.tile([T, DL[0]], f32, tag="zp")
                for c in range(2):
                    nc.tensor.matmul(
                        zp[:, :],
                        xt_sb[:, c * SHARD + j * T: c * SHARD + (j + 1) * T],
                        w1_sb[:, c * DL[0]:(c + 1) * DL[0]],
                        start=(c == 0), stop=(c == 1),
                    )
                nc.vector.tensor_scalar(
                    zb1[j][:, :], zp[:, :], dis_sb[:, j:j + 1], None, mult)
                z_store(0, j, zb1[j])
                if j == HT_A - 1:
                    issue_ag(0, 0)
            issue_ag(0, 1)

            # ---- L1 gather helper ----
            gq = [0]

            def gather_group(h, j, gslot):
                cnt = CNT[h][j]
                nb = nbl[h][j]
                gt3 = gbuf[gslot][:, :nb * DL[0]].rearrange(
                    "p (n d) -> p n d", d=DL[0])
                g = nc.gpsimd.dma_gather(
                    gt3,
                    agout[0][h].ap(),
                    idx_sb[:, ioff[h][j]:ioff[h][j] + cnt // 16],
                    cnt, cnt, DL[0],
                    single_packet=False,
                    queue_num=gq[0] % 4,
                )
                gq[0] += 1
                add_dep_helper(g.ins, ag_insts[0][h].ins, reason="g ag")
                st = selpool.tile([T, maxnb * T], fp8, tag="sel")
                nc.sync.dma_start(
                    out=st[:, :nb * T],
                    in_=sel[:, boff[h][j] * T:(boff[h][j] + nb) * T])
                return gt3, st, nb

            # ---- cold gathers / matmuls for L2/L3 ----
            def cold_gathers(l):
                for j in range(NT):
                    cnt = CNT2[j]
                    gt3 = cbuf[j][:, :nbl2[j] * DPAD[l]].rearrange(
                        "p (n d) -> p n d", d=DPAD[l])
                    g = nc.gpsimd.dma_gather(
                        gt3,
                        agout[l][2].ap(),
                        idx2_sb[:, ioff2[j]:ioff2[j] + cnt // 16],
                        cnt, cnt, DPAD[l],
                        single_packet=False,
                        queue_num=gq[0] % 4,
                    )
                    gq[0] += 1
                    add_dep_helper(g.ins, ag_insts[l][2].ins, reason="cg ag")

            def cold_matmuls_chunk(l, c):
                """Fold cold edges of dst tiles 4c..4c+3 into fm[c]; the last
                one closes the accumulation group."""
                d_el = DL[l]
                for j in range(4 * c, 4 * c + 4):
                    nb = nbl2[j]
                    gt3 = cbuf[j][:, :nb * DPAD[l]].rearrange(
                        "p (n d) -> p n d", d=DPAD[l])
                    st = selpool.tile([T, maxnb * T], fp8, tag="sel")
                    nc.sync.dma_start(
                        out=st[:, :nb * T],
                        in_=sel2[:, boff2[j] * T:(boff2[j] + nb) * T])
                    r = (j % 4) * T
                    for b in range(nb):
                        nc.tensor.matmul(
                            fm[c][:d_el, r:r + T],
                            gt3[:, b, :d_el],
                            st[:, b * T:(b + 1) * T],
                            start=False,
                            stop=(j % 4 == 3 and b == nb - 1),
                            skip_group_check=True)

            # ---- hot scatter: chunk-a tiles then per-core b1 stripes; S
            # fetched two tiles per DMA on the Activation HWDGE queue. ----
            fetch_plan = []           # (smat_row0, ntiles, ag chunk)
            for g2 in range(SA_TILES // 2):
                fetch_plan.append((2 * g2, 2, 0))
            for core in range(N_CORES):
                base = SA_TILES + core * HOTB
                k = 0
                while k < HOTB:
                    n = min(2, HOTB - k)
                    fetch_plan.append((base + k, n, 1))
                    k += n
            hot_steps = []            # (fetch_idx, k_in_fetch)
            for fi, (r0, n, ch) in enumerate(fetch_plan):
                for k in range(n):
                    hot_steps.append((fi, k))
            n_hot = len(hot_steps)

            scat = {"pos": 0, "zsb": None, "stile": None}

            def scatter_steps(l, n, limit):
                d_el = DL[l]
                dp = DPAD[l]
                while n > 0 and scat["pos"] < limit:
                    pos = scat["pos"]
                    fi, k = hot_steps[pos]
                    r0, fn, ch = fetch_plan[fi]
                    if pos < SA_TILES:
                        if pos % 8 == 0:
                            zsb = zspool.tile([T, 8 * dp], bf16,
                                              tag=f"zsa{l}")
                            d = nc.sync.dma_start(
                                out=zsb.rearrange("p (n d) -> p n d", d=dp),
                                in_=agout_v[l][0][:, pos:pos + 8, :])
                            add_dep_helper(d.ins, ag_insts[l][0].ins,
                                           reason="zs ag")
                            scat["zsb"] = zsb
                        zk = pos % 8
                    else:
                        p = pos - SA_TILES
                        if p % HOTB == 0:
                            core = p // HOTB
                            zsb = zspool.tile([T, HOTB * dp], bf16,
                                              tag=f"zsb{l}")
                            d = nc.sync.dma_start(
                                out=zsb.rearrange("p (n d) -> p n d", d=dp),
                                in_=agout_v[l][1][:, core * HOTB:
                                                  (core + 1) * HOTB, :])
                            add_dep_helper(d.ins, ag_insts[l][1].ins,
                                           reason="zs ag")
                            scat["zsb"] = zsb
                        zk = p % HOTB
                    if k == 0:
                        stile = sbpool.tile([T, 2 * SHARD], fp8, tag="sm")
                        nc.scalar.dma_start(
                            out=stile[:, :fn * SHARD].rearrange(
                                "p (n d) -> p n d", d=SHARD),
                            in_=smat_v[:, r0:r0 + fn, :])
                        scat["stile"] = stile
                    stile = scat["stile"]
                    zsb = scat["zsb"]
                    for c in range(NCH):
                        nc.tensor.matmul(
                            fm[c][:d_el, :],
                            zsb[:, zk * dp:zk * dp + d_el],
                            stile[:, k * SHARD + c * CCHUNK:
                                  k * SHARD + (c + 1) * CCHUNK],
                            start=(pos == 0), stop=False,
                            skip_group_check=True)
                    scat["pos"] = pos + 1
                    n -= 1

            # ================= L1 phase A (src chunk a) ====================
            for j in range(NT):
                gt3, st, nb = gather_group(0, j, j % 3)
                ps = ps_agg.tile([T, DL[0]], f32, tag="agg")
                nc.tensor.matmul(ps[:, :], identb_sb[:, :], zb1[j][:, :],
                                 start=True, stop=False)
                for b in range(nb):
                    nc.tensor.matmul(
                        ps[:, :], st[:, b * T:(b + 1) * T], gt3[:, b, :],
                        start=False, stop=(b == nb - 1))
                nc.scalar.activation(
                    acc[:, j * DL[0]:(j + 1) * DL[0]], ps[:, :], fcopy)

            # ================= L1 phase B + post + Z2 + AG2 ================
            for j in range(NT):
                gt3, st, nb = gather_group(1, j, j % 3)
                ps = ps_agg.tile([T, DL[0]], f32, tag="agg")
                for b in range(nb):
                    nc.tensor.matmul(
                        ps[:, :], st[:, b * T:(b + 1) * T], gt3[:, b, :],
                        start=(b == 0), stop=(b == nb - 1))
                u = tpool.tile([T, DL[0]], f32, tag="post")
                nc.vector.tensor_tensor(
                    u[:, :], ps[:, :], acc[:, j * DL[0]:(j + 1) * DL[0]], add)
                if apply_b1:
                    u2 = tpool.tile([T, DL[0]], f32, tag="post")
                    nc.vector.tensor_tensor(u2[:, :], u[:, :], brep1_sb[:, :],
                                            add)
                    u = u2
                h1 = hpool.tile([T, DL[0]], bf16, tag="h1")
                nc.scalar.activation(h1[:, :], u[:, :], relu,
                                     scale=dis_sb[:, j:j + 1])
                zp2 = ps_z.tile([T, DL[0]], f32, tag="zp")
                for c in range(2):
                    tp = ps_t.tile([T, T], bf16, tag="tp")
                    nc.tensor.matmul(tp[:, :], h1[:, c * T:(c + 1) * T],
                                     identb_sb[:, :], is_transpose=True)
                    htc = htpool.tile([T, T], bf16, tag="ht")
                    nc.scalar.activation(htc[:, :], tp[:, :], fcopy)
                    nc.tensor.matmul(
                        zp2[:, :DL[1]], htc[:, :],
                        w2_sb[:, c * DL[1]:(c + 1) * DL[1]],
                        start=(c == 0), stop=(c == 1))
                zb2 = zbpool.tile([T, DL[1]], bf16, tag="zb2")
                nc.vector.tensor_scalar(
                    zb2[:, :], zp2[:, :DL[1]], dis_sb[:, j:j + 1], None, mult)
                z_store(1, j, zb2)
                if j == HT_A - 1:
                    issue_ag(1, 0)
                if j == JCUT - 1:
                    issue_ag(1, 1)
                if j == NT - 1:
                    issue_ag(1, 2)
                if j >= HT_A:
                    lim = SA_TILES if j < JCUT else n_hot
                    scatter_steps(1, 3, lim)

            # ---- layer tails: finish aggregation, pipeline per chunk ----
            def layer_tail(l):
                last = l == 2
                cold_gathers(l)
                scatter_steps(l, n_hot, n_hot)
                for c in range(NCH):
                    cold_matmuls_chunk(l, c)
                    d_el = DL[l]
                    t = tpool.tile([T, CCHUNK], f32, tag="fmpost")
                    nc.vector.tensor_tensor(
                        t[:d_el, :], fm[c][:d_el, :],
                        disrow_sb[:d_el, c * CCHUNK:(c + 1) * CCHUNK], mult)
                    if not last:
                        nc.scalar.activation(
                            h2fm[:, c * CCHUNK:(c + 1) * CCHUNK],
                            t[:d_el, :], relu, bias=b2_sb[:, :])
                        for j in range(4 * c, 4 * c + 4):
                            zp3 = ps_z.tile([T, DL[0]], f32, tag="zp")
                            nc.tensor.matmul(
                                zp3[:, :DL[2]], h2fm[:, j * T:(j + 1) * T],
                                w3_sb[:, :], start=True, stop=True)
                            zb3 = zbpool.tile([T, DPAD[2]], bf16, tag="zb3")
                            nc.vector.tensor_scalar(
                                zb3[:, :DL[2]], zp3[:, :DL[2]],
                                dis_sb[:, j:j + 1], None, mult)
                            z_store(2, j, zb3)
                            if j == HT_A - 1:
                                issue_ag(2, 0)
                            if j == JCUT - 1:
                                issue_ag(2, 1)
                            if j == NT - 1:
                                issue_ag(2, 2)
                    else:
                        if apply_b3:
                            nc.scalar.activation(
                                outfm[:DL[2], c * CCHUNK:(c + 1) * CCHUNK],
                                t[:DL[2], :], fcopy, bias=b3_sb[:DL[2], :])
                        else:
                            nc.scalar.activation(
                                outfm[:DL[2], c * CCHUNK:(c + 1) * CCHUNK],
                                t[:DL[2], :], fcopy)
                        for j in range(4 * c, 4 * c + 4):
                            tpf = ps_z.tile([T, DL[0]], f32, tag="zp")
                            nc.tensor.matmul(
                                tpf[:, :DL[2]],
                                outfm[:DL[2], j * T:(j + 1) * T],
                                identf_sb[:DL[2], :DL[2]],
                                is_transpose=True)
                            ot = htpool.tile([T, DL[2]], f32, tag="ot")
                            nc.scalar.activation(ot[:, :], tpf[:, :DL[2]],
                                                 fcopy)
                            nc.sync.dma_start(out=out_v[:, j, :],
                                              in_=ot[:, :])

            layer_tail(1)
            scat["pos"] = 0
            layer_tail(2)

    nc.compile()
    return nc


# ----------------------------------------------------------------------------
# Host-side preprocessing
# ----------------------------------------------------------------------------
def _band_node_order(outdeg, indeg):
    by_out = np.argsort(-outdeg, kind="stable")
    node_order = -np.ones(NTAB, np.int64)
    new_pos = np.zeros(N_NODES, np.int64)
    band_sz = N_CORES * T
    for k in range(NT):
        band = by_out[k * band_sz:(k + 1) * band_sz]
        band = band[np.argsort(-indeg[band], kind="stable")]
        fill = np.zeros(N_CORES, np.int64)
        b = 0
        direction = 1
        for node in band:
            pos = b * SHARD + k * T + fill[b]
            node_order[pos] = node
            new_pos[node] = pos
            fill[b] += 1
            b += direction
            if b == N_CORES:
                b = N_CORES - 1
                direction = -1
            elif b < 0:
                b = 0
                direction = 1
    return node_order, new_pos


def _group_pack(core_s, grp, ngrp, row_s, slot_s, CNT_flat, ioff_flat,
                boff_flat):
    grp_start = np.zeros(N_CORES * ngrp + 1, np.int64)
    np.add.at(grp_start, core_s * ngrp + grp + 1, 1)
    grp_start = np.cumsum(grp_start)
    rank = np.arange(len(grp)) - grp_start[core_s * ngrp + grp]
    cnt_np = np.array(CNT_flat)
    ioff_np = np.array(ioff_flat)
    boff_np = np.array(boff_flat)
    epos = ioff_np[grp] * 16 + rank
    blk = boff_np[grp] + rank // T
    lane = rank % T
    idxcols = int(ioff_np[-1] + cnt_np[-1] // 16)
    totblk = int(boff_np[-1] + (cnt_np[-1] + T - 1) // T)
    idx_cores, sel_cores = [], []
    for c in range(N_CORES):
        m = core_s == c
        flat = np.zeros(idxcols * 16, np.int16)
        flat[epos[m]] = row_s[m].astype(np.int16)
        wrapped = np.tile(flat.reshape(idxcols, 16).T, (8, 1))
        idx_cores.append(np.ascontiguousarray(wrapped.astype(np.int16)))
        selc = np.zeros((totblk, T, T), np.uint8)
        selc[blk[m], lane[m], slot_s[m]] = 1
        sel_cores.append(np.ascontiguousarray(
            selc.transpose(1, 0, 2).reshape(T, totblk * T)).astype(FP8))
    return idx_cores, sel_cores


def _preprocess(edge_index):
    src = np.asarray(edge_index[0], dtype=np.int64)
    dst = np.asarray(edge_index[1], dtype=np.int64)
    indeg = np.bincount(dst, minlength=N_NODES).astype(np.float64) + 1.0
    outdeg = np.bincount(src, minlength=N_NODES).astype(np.float64)
    dis_full = 1.0 / np.sqrt(indeg)

    node_order, new_pos = _band_node_order(outdeg, indeg)

    spos = new_pos[src]
    dpos = new_pos[dst]
    core = dpos // SHARD
    tile = (dpos % SHARD) // T
    slot = dpos % T
    shalf = ((spos % SHARD) >= HALF_A).astype(np.int64)
    srow_half = ((spos // SHARD) * np.where(shalf == 0, HALF_A, HALF_B)
                 + (spos % SHARD) - shalf * HALF_A)

    counts = np.zeros((N_CORES, 2, NT), np.int64)
    np.add.at(counts, (core, shalf, tile), 1)
    CNT = [[max(16, _ru16(counts[:, h, j].max())) for j in range(NT)]
           for h in range(2)]
    CNT_flat = [c for row in CNT for c in row]
    _, boff_f, ioff_f, _, _ = _offsets(CNT)

    order = np.lexsort((slot, tile, shalf, core))
    g1 = shalf[order] * NT + tile[order]
    idx_cores, sel_cores = _group_pack(
        core[order], g1, 2 * NT, srow_half[order], slot[order],
        CNT_flat, ioff_f, boff_f)

    # ---- cold edges (src tile >= JCUT -> AG chunk 2) for L2/L3 ----
    loop_pos = new_pos[node_order[node_order >= 0]]
    s_all = np.concatenate([spos, loop_pos])
    d_all = np.concatenate([dpos, loop_pos])
    stile_all = (s_all % SHARD) // T
    cold_m = stile_all >= JCUT
    sc = s_all[cold_m]
    dc = d_all[cold_m]
    ccore = dc // SHARD
    ctile = (dc % SHARD) // T
    cslot = dc % T
    crow = (sc // SHARD) * (NCOLD * T) + (sc % SHARD) - JCUT * T
    counts2 = np.zeros((N_CORES, NT), np.int64)
    np.add.at(counts2, (ccore, ctile), 1)
    CNT2 = [max(16, _ru16(counts2[:, j].max())) for j in range(NT)]
    _, boff2_f, ioff2_f, _, _ = _offsets([CNT2])
    order2 = np.lexsort((cslot, ctile, ccore))
    idx2_cores, sel2_cores = _group_pack(
        ccore[order2], ctile[order2], NT, crow[order2], cslot[order2],
        CNT2, ioff2_f, boff2_f)

    # ---- S matrices: hot srcs only, rows in [a | b1-hot] order ----
    hot_m = ~cold_m
    sh = s_all[hot_m]
    dh = d_all[hot_m]
    sh_tile = (sh % SHARD) // T
    sh_core = sh // SHARD
    sh_off = sh % T
    in_a = sh_tile < HT_A
    srow_glob = np.where(
        in_a,
        sh_core * HALF_A + sh_tile * T + sh_off,
        N_CORES * HALF_A + sh_core * (HOTB * T)
        + (sh_tile - HT_A) * T + sh_off)
    dcore_h = dh // SHARD
    dloc_h = dh % SHARD
    smat_cores = []
    for c in range(N_CORES):
        m = dcore_h == c
        S = np.zeros((NTAB, SHARD), np.uint8)
        np.add.at(S, (srow_glob[m], dloc_h[m]), 1)
        smat_cores.append(S.astype(FP8))

    dis_cores, disrow_cores = [], []
    for c in range(N_CORES):
        slots = node_order[c * SHARD:(c + 1) * SHARD]
        dis_c = np.where(slots >= 0, dis_full[np.maximum(slots, 0)], 0.0)
        dis_cores.append(np.ascontiguousarray(
            dis_c.reshape(NT, T).T).astype(np.float32))
        disrow_cores.append(np.ascontiguousarray(
            np.tile(dis_c[None, :], (T, 1))).astype(np.float32))

    return (idx_cores, sel_cores, idx2_cores, sel2_cores, dis_cores,
            disrow_cores, smat_cores, CNT, CNT2, node_order)


def _make_in_maps(x, W1, b1, W2, b2, W3, b3, edge_index):
    (idx_cores, sel_cores, idx2_cores, sel2_cores, dis_cores, disrow_cores,
     smat_cores, CNT, CNT2, node_order) = _preprocess(edge_index)

    x = np.asarray(x, np.float32)
    w1b = np.asarray(W1, np.float32).astype(BF16)
    w2b = np.asarray(W2, np.float32).astype(BF16)
    w3b = np.asarray(W3, np.float32).astype(BF16)
    b1f = np.asarray(b1, np.float32)
    b2f = np.asarray(b2, np.float32)
    b3f = np.asarray(b3, np.float32)
    brep1 = np.tile(b1f, (T, 1))
    b2col = np.zeros((T, 1), np.float32)
    b2col[:DL[1], 0] = b2f
    b3col = np.zeros((T, 1), np.float32)
    b3col[:DL[2], 0] = b3f
    identb = np.eye(T, dtype=BF16)
    identf = np.eye(T, dtype=np.float32)
    apply_b1 = bool(np.any(b1f))
    apply_b3 = bool(np.any(b3f))

    in_maps = []
    for c in range(N_CORES):
        slots = node_order[c * SHARD:(c + 1) * SHARD]
        xs = np.where((slots >= 0)[:, None], x[np.maximum(slots, 0)], 0.0)
        in_maps.append({
            "xt": np.ascontiguousarray(xs.T.astype(np.float32)).astype(BF16),
            "w1": w1b, "w2": w2b, "w3": w3b,
            "brep1": brep1, "b2col": b2col, "b3col": b3col,
            "dis": dis_cores[c], "disrow": disrow_cores[c],
            "identb": identb, "identf": identf,
            "idx": idx_cores[c], "sel": sel_cores[c],
            "idx2": idx2_cores[c], "sel2": sel2_cores[c],
            "smat": smat_cores[c],
        })
    return in_maps, CNT, CNT2, node_order, apply_b1, apply_b3


_NC_CACHE = {}


def kernel_with_results(x, W1, b1, W2, b2, W3, b3, edge_index, trace=False):
    in_maps, CNT, CNT2, node_order, apply_b1, apply_b3 = _make_in_maps(
        x, W1, b1, W2, b2, W3, b3, edge_index)
    key = (tuple(CNT[0]), tuple(CNT[1]), tuple(CNT2), apply_b1, apply_b3)
    if key not in _NC_CACHE:
        _NC_CACHE[key] = _build_nc(CNT, CNT2, apply_b1, apply_b3)
    nc = _NC_CACHE[key]
    res = run_bass_kernel_spmd(
        nc, in_maps, core_ids=list(range(N_CORES)), trace=trace)
    rows = np.concatenate(
        [np.asarray(res.results[c]["out"]) for c in range(N_CORES)], axis=0)
    full = np.zeros((N_NODES, rows.shape[1]), np.float32)
    real = node_order >= 0
    full[node_order[real]] = rows[real]
    return full, res


def kernel(x, W1, b1, W2, b2, W3, b3, edge_index):
    full, _ = kernel_with_results(x, W1, b1, W2, b2, W3, b3, edge_index)
    return full
